# revision 1
# baseline (speedup 1.0000x reference)
"""Trainium2 Bass kernel for nn_MMN_7361573945989 (MatchNet corr/attention).

Math (per batch b):
  qn_l = l2norm_c(fq_l); sn_l = l2norm_c(fs_l)           l in {4, 3}
  logits[p, q] = TEMP * (w0 * qn4.T@sn4 + w1 * qn3.T@sn3)[p, q]
  attn = softmax_q(logits)
  att_fq[c, p] = sum_q attn[p, q] * f_s[c, q]
  fq_out = l2norm_c(f_q) + l2norm_c(att_fq) * ATT_WT
  returns (fq_out, att_fq)

Sharding: 8 cores = 2 batches x 4 query-pixel shards of 900.

Per-core kernel (transposed orientation, logits live as [q, p] tiles):
  - channel-norm sums of squares: ACT squares -> GpSimd reduce over channel
    groups -> one ones-vector matmul per layer (contracts the partition dim)
  - row-vector [1,n] values are broadcast across partitions via K=1 matmuls
    FIRST, then sqrt/reciprocal run as full-width [128,n] ops (a [1,n] op
    uses a single lane and is ~10x slower)
  - normalization scales and TEMP*w fold into the bf16 matmul operands, so
    logits accumulate in one PSUM group; lhsT is reused across both p-blocks
    (2 matmuls per LDWEIGHTS)
  - softmax without max-subtraction: logits = 20*(w.cos) are bounded
  - softmax denominators accumulate during phase A via ones-matmuls over the
    exp tiles; phase B computes Y = exp @ f_s.T directly in the [c, p] output
    orientation (2 matmuls per LDWEIGHTS via paired p-blocks), then scales by
    1/denom; the att_fq l2norm uses ||Y|| so the denominator cancels
  - f_s arrives pre-transposed from the host ([hw, cv]) so no PE transposes
"""

import sys
from contextlib import ExitStack

import numpy as np

sys.path.insert(0, "/opt/trn_rl_repo")

import concourse.bass as bass  # noqa: E402
import concourse.tile as tile  # noqa: E402
from concourse import mybir  # noqa: E402
from concourse.bass_utils import run_bass_kernel_spmd  # noqa: E402

B, H, W = 2, 60, 60
HW = H * W  # 3600
C3, C4, CV = 1024, 2048, 512
TEMP = 20.0
ATT_WT = 0.3
NCORES = 8
PSH = 4  # query-pixel shards per batch
P = HW // PSH  # 900 query pixels per core
PB = P // 2  # 450, p-block (one PSUM bank of fp32)
NQC = (HW + 127) // 128  # 29 support-pixel chunks
QT = HW - (NQC - 1) * 128  # 16 rows in the tail chunk
NC4, NC3, NCV = C4 // 128, C3 // 128, CV // 128  # 16, 8, 4
NCI = NC4 + NC3  # 24 combined channel chunks

F32 = mybir.dt.float32
BF16 = mybir.dt.bfloat16
AF = mybir.ActivationFunctionType
MUL = mybir.AluOpType.mult

_MAX_WAITS_PER_INST = 1


def _patched_drain_and_barrier(self, tick_clock, wait_clock):
    """Tile's kernel-tail drain carries one sem wait per engine/queue; the
    walrus build used here accepts only one sync wait per CTRL instruction.
    Split the waits across extra sync-engine nops."""
    drain_inst = self.nc.sync.drain()
    wait_clock.add_sem_waits(
        drain_inst.ins, tile.ScopedClock({None: tick_clock.global_clock})
    )
    si = drain_inst.ins.sync_info
    if si is not None and len(si.on_wait) > _MAX_WAITS_PER_INST:
        waits = list(si.on_wait)
        drain_inst.ins.sync_info = mybir.SyncInfo(
            on_wait=waits[:_MAX_WAITS_PER_INST], on_update=list(si.on_update)
        )
        for i in range(_MAX_WAITS_PER_INST, len(waits), _MAX_WAITS_PER_INST):
            nop = self.nc.sync.nop()
            nop.ins.sync_info = mybir.SyncInfo(
                on_wait=waits[i : i + _MAX_WAITS_PER_INST], on_update=[]
            )
    self.nc.all_engine_barrier()
    assert self.sems is not None
    popped = self.nc._tile_sem_poison_stack.pop()
    assert popped is self._sem_poison
    self.nc.clear_and_free_semaphores(list(self.sems.allocated().values()))
    self.nc.all_engine_barrier()


tile.TileContext._drain_and_barrier = _patched_drain_and_barrier


def _split_sync_waits(nc, max_waits=_MAX_WAITS_PER_INST):
    """Walrus here accepts at most one sync wait per instruction; move excess
    waits onto same-engine nops inserted immediately before the instruction."""
    ctr = 0
    for f in nc.m.functions:
        for blk in f.blocks:
            insts = list(blk.instructions)
            out = []
            changed = False
            for inst in insts:
                si = inst.sync_info
                if si is not None and len(si.on_wait) > max_waits:
                    waits = list(si.on_wait)
                    for i0 in range(max_waits, len(waits), max_waits):
                        ctr += 1
                        nop = mybir.InstNoOp(
                            name=f"waitsplit-{ctr}",
                            engine=inst.engine,
                            bass_nofuse=True,
                            sync_info=mybir.SyncInfo(
                                on_wait=waits[i0 : i0 + max_waits], on_update=[]
                            ),
                        )
                        nc.register_instruction(nop, overwrite=True)
                        out.append(nop)
                    inst.sync_info = mybir.SyncInfo(
                        on_wait=waits[:max_waits], on_update=list(si.on_update)
                    )
                    changed = True
                out.append(inst)
            if changed:
                blk.instructions = out


def build():
    nc = bass.Bass()
    q4 = nc.dram_tensor("q4", [C4, P], F32, kind="ExternalInput")
    q3 = nc.dram_tensor("q3", [C3, P], F32, kind="ExternalInput")
    s4 = nc.dram_tensor("s4", [C4, HW], F32, kind="ExternalInput")
    s3 = nc.dram_tensor("s3", [C3, HW], F32, kind="ExternalInput")
    vt = nc.dram_tensor("vt", [HW, CV], F32, kind="ExternalInput")  # f_s.T
    fq = nc.dram_tensor("fq", [CV, P], F32, kind="ExternalInput")
    wv = nc.dram_tensor("wv", [1, 2], F32, kind="ExternalInput")  # [T*w0, T*w1]
    att_o = nc.dram_tensor("att_o", [CV, P], F32, kind="ExternalOutput")
    fq_o = nc.dram_tensor("fq_o", [CV, P], F32, kind="ExternalOutput")

    def load_blocks(dst, dst_cols, ci0, src, col0, ncols, n_ci, group=4):
        """Load `n_ci` row-blocks of 128 from DRAM `src` (cols [col0,col0+ncols))
        into SBUF tile `dst` whose free layout is (ci, dst_cols)."""
        srcr = src[:].rearrange("(ci c) x -> c ci x", c=128)
        dstr = dst[:].rearrange("c (ci x) -> c ci x", x=dst_cols)
        for g0 in range(0, n_ci, group):
            g = min(group, n_ci - g0)
            nc.sync.dma_start(
                dstr[:, ci0 + g0 : ci0 + g0 + g, 0:ncols],
                srcr[:, g0 : g0 + g, col0 : col0 + ncols],
            )

    with tile.TileContext(nc) as tc:
        with ExitStack() as octx:
            cpool = octx.enter_context(tc.tile_pool(name="const", bufs=1))
            ones_col = cpool.tile([128, 1], BF16)
            nc.gpsimd.memset(ones_col[:], 1.0)
            ones_row = cpool.tile([1, 128], F32)
            nc.gpsimd.memset(ones_row[:], 1.0)
            ones_col_f = cpool.tile([128, 1], F32)
            nc.gpsimd.memset(ones_col_f[:], 1.0)
            w_sb = cpool.tile([1, 2], F32)
            nc.sync.dma_start(w_sb[:], wv[:])
            w_col = cpool.tile([128, 2], F32)

            pers = octx.enter_context(tc.tile_pool(name="pers", bufs=1))
            qns = pers.tile([128, NCI * P], BF16)  # scaled query feats (ci, p)
            fqn = pers.tile([128, NCV * P], F32)  # normalized f_q (ci, p)
            expT = pers.tile([128, NQC * P], BF16)  # exp(logits) (qc; q, p)
            # zero the tail-chunk region so K=128 matmuls over the tail are
            # exact (rows [0:QT] get real data later)
            nc.gpsimd.memset(expT[:, (NQC - 1) * P : NQC * P], 0.0)

            dnps = octx.enter_context(
                tc.tile_pool(name="dnps", bufs=1, space="PSUM")
            )
            dns = [
                dnps.tile([1, PB], F32, tag=f"dn{pb}", name=f"dn{pb}")
                for pb in range(2)
            ]

            # broadcast T*w across partitions once: [1,2] -> [128,2]
            with tc.tile_pool(name="wps", bufs=1, space="PSUM") as wps:
                w_ps = wps.tile([128, 2], F32)
                nc.tensor.matmul(w_ps[:], ones_row[:], w_sb[:])
                nc.scalar.copy(w_col[:], w_ps[:])

            # ---------------- prep: query-side normalization ----------------
            with ExitStack() as pctx:
                xpool = pctx.enter_context(tc.tile_pool(name="prepx", bufs=2))
                sqpool = pctx.enter_context(tc.tile_pool(name="prepsq", bufs=2))
                mini = pctx.enter_context(tc.tile_pool(name="prepmini", bufs=2))
                pps = pctx.enter_context(
                    tc.tile_pool(name="prepps", bufs=1, space="PSUM")
                )

                layers = [
                    (q4, NC4, qns, 0),
                    (q3, NC3, qns, NC4),
                    (fq, NCV, fqn, 0),
                ]
                for src, n_ci, dst, ci0 in layers:
                    ss = [
                        pps.tile([1, PB], F32, tag=f"ss{pb}", name=f"ss{pb}")
                        for pb in range(2)
                    ]
                    for g0 in range(0, n_ci, 4):
                        g = min(4, n_ci - g0)
                        if dst is fqn:
                            load_blocks(fqn, P, g0, src, 0, P, g)
                            xg = fqn[:, g0 * P : (g0 + g) * P]
                        else:
                            xt = xpool.tile([128, 4 * P], F32, tag="x")
                            load_blocks(xt, P, 0, src[g0 * 128 :, :], 0, P, g)
                            xg = xt[:, 0 : g * P]
                            nc.vector.tensor_copy(
                                dst[:, (ci0 + g0) * P : (ci0 + g0 + g) * P], xg
                            )
                        for k in range(g):
                            ci = g0 + k
                            sq = sqpool.tile([128, P], BF16, tag="sq")
                            nc.scalar.square(sq[:], xg[:, k * P : (k + 1) * P])
                            for pb in range(2):
                                nc.tensor.matmul(
                                    ss[pb][:],
                                    ones_col[:],
                                    sq[:, pb * PB : (pb + 1) * PB],
                                    start=(ci == 0),
                                    stop=(ci == n_ci - 1),
                                )
                    for pb in range(2):
                        # broadcast-first: [1,PB] -> [128,PB], then full-width
                        # sqrt + reciprocal
                        u = mini.tile([1, PB], F32, tag="u")
                        nc.scalar.copy(u[:], ss[pb][:])
                        bc = pps.tile(
                            [128, PB], F32, tag=f"bc{pb}", name=f"bc{pb}"
                        )
                        nc.tensor.matmul(bc[:], ones_row[:], u[:])
                        st = mini.tile([128, PB], F32, tag="st")
                        nc.scalar.sqrt(st[:], bc[:])
                        ninv = mini.tile([128, PB], F32, tag="ninv")
                        nc.vector.reciprocal(ninv[:], st[:])
                        for ci in range(n_ci):
                            sl = slice(
                                (ci0 + ci) * P + pb * PB,
                                (ci0 + ci) * P + pb * PB + PB,
                            )
                            nc.vector.tensor_mul(dst[:, sl], dst[:, sl], ninv[:])

            # ------------- main: support stream, logits, exp, denom -------------
            with ExitStack() as mctx:
                snpool = mctx.enter_context(tc.tile_pool(name="sn", bufs=2))
                snspool = mctx.enter_context(tc.tile_pool(name="sns", bufs=3))
                msq = mctx.enter_context(tc.tile_pool(name="msq", bufs=2))
                mpart = mctx.enter_context(tc.tile_pool(name="mpart", bufs=2))
                mmini = mctx.enter_context(tc.tile_pool(name="mmini", bufs=2))
                lps = mctx.enter_context(
                    tc.tile_pool(name="logits", bufs=1, space="PSUM")
                )
                sps = mctx.enter_context(
                    tc.tile_pool(name="snps", bufs=1, space="PSUM")
                )

                for qc in range(NQC):
                    qn = 128 if qc < NQC - 1 else QT
                    sn_sb = snpool.tile([128, NCI * 128], F32, tag="sn")
                    load_blocks(sn_sb, 128, 0, s4, qc * 128, qn, NC4)
                    load_blocks(sn_sb, 128, NC4, s3, qc * 128, qn, NC3)

                    # squares (ACT), group-reduce over ci (GpSimd), then one
                    # ones-matmul per layer to contract the partition dim
                    sq = msq.tile([128, NCI * 128], BF16, tag="sq")
                    for g0 in range(0, NCI, 4):
                        if qn == 128:
                            nc.scalar.square(
                                sq[:, g0 * 128 : (g0 + 4) * 128],
                                sn_sb[:, g0 * 128 : (g0 + 4) * 128],
                            )
                        else:
                            for k in range(4):
                                nc.scalar.square(
                                    sq[:, (g0 + k) * 128 : (g0 + k) * 128 + qn],
                                    sn_sb[:, (g0 + k) * 128 : (g0 + k) * 128 + qn],
                                )
                    sqv = sq[:].rearrange("c (ci q) -> c q ci", ci=NCI)
                    bcs = []
                    for ln, lo, n_ci in ((0, 0, NC4), (1, NC4, NC3)):
                        part = mpart.tile([128, 128], F32, tag=f"part{ln}")
                        nc.vector.reduce_sum(
                            part[:, 0:qn],
                            sqv[:, 0:qn, lo : lo + n_ci],
                            axis=mybir.AxisListType.X,
                        )
                        ssl = sps.tile(
                            [1, 128], F32, tag=f"ss{ln}", name=f"ss{ln}"
                        )
                        nc.tensor.matmul(
                            ssl[:, 0:qn], ones_col_f[:], part[:, 0:qn]
                        )
                        u = mmini.tile([1, 128], F32, tag=f"u{ln}")
                        nc.scalar.copy(u[:, 0:qn], ssl[:, 0:qn])
                        bcp = sps.tile(
                            [128, 128], F32, tag=f"bc{ln}", name=f"bc{ln}"
                        )
                        nc.tensor.matmul(
                            bcp[:, 0:qn], ones_row[:], u[:, 0:qn]
                        )
                        st = mmini.tile([128, 128], F32, tag=f"st{ln}")
                        nc.scalar.sqrt(st[:, 0:qn], bcp[:, 0:qn])
                        ninv = mmini.tile([128, 128], F32, tag=f"ninv{ln}")
                        nc.vector.reciprocal(ninv[:, 0:qn], st[:, 0:qn])
                        bcs.append(ninv)
                    sn_s = snspool.tile([128, NCI * 128], BF16, tag="sns")
                    for ci in range(NCI):
                        ln = 0 if ci < NC4 else 1
                        nc.vector.scalar_tensor_tensor(
                            sn_s[:, ci * 128 : ci * 128 + qn],
                            sn_sb[:, ci * 128 : ci * 128 + qn],
                            w_col[:, ln : ln + 1],
                            bcs[ln][:, 0:qn],
                            MUL,
                            MUL,
                        )

                    # logits: lhsT reused across both p-blocks (2 MM / LDW)
                    ps0 = lps.tile([128, PB], F32, tag="logits0", name="l0")
                    ps1 = lps.tile([128, PB], F32, tag="logits1", name="l1")
                    for ci in range(NCI):
                        lhsT = sn_s[:, ci * 128 : ci * 128 + qn]
                        for pb, ps in ((0, ps0), (1, ps1)):
                            nc.tensor.matmul(
                                ps[0:qn, :],
                                lhsT,
                                qns[:, ci * P + pb * PB : ci * P + pb * PB + PB],
                                start=(ci == 0),
                                stop=(ci == NCI - 1),
                            )
                    for pb, ps in ((0, ps0), (1, ps1)):
                        esl = expT[
                            0:qn, qc * P + pb * PB : qc * P + pb * PB + PB
                        ]
                        nc.scalar.activation(esl, ps[0:qn, :], AF.Exp)
                        # softmax denominator accumulates across all chunks
                        nc.tensor.matmul(
                            dns[pb][:],
                            ones_col[:],
                            expT[:, qc * P + pb * PB : qc * P + pb * PB + PB],
                            start=(qc == 0),
                            stop=(qc == NQC - 1),
                        )

            # ---------------- phase B: attention-weighted values ----------------
            with ExitStack() as bctx:
                vpool = bctx.enter_context(tc.tile_pool(name="vtp", bufs=1))
                vstage = bctx.enter_context(tc.tile_pool(name="vst", bufs=3))
                yps = bctx.enter_context(
                    tc.tile_pool(name="yps", bufs=1, space="PSUM")
                )
                bps = bctx.enter_context(
                    tc.tile_pool(name="bps", bufs=1, space="PSUM")
                )
                bsq = bctx.enter_context(tc.tile_pool(name="bsq", bufs=2))
                bmini = bctx.enter_context(tc.tile_pool(name="bmini", bufs=1))
                batt = bctx.enter_context(tc.tile_pool(name="batt", bufs=1))
                bout = bctx.enter_context(tc.tile_pool(name="bout", bufs=2))

                # stream f_s.T, cast to bf16 (tail rows zeroed for K=128 MMs)
                vt_all = vpool.tile([128, NQC * CV], BF16)
                nc.gpsimd.memset(vt_all[:, (NQC - 1) * CV : NQC * CV], 0.0)
                for qc in range(NQC):
                    qn = 128 if qc < NQC - 1 else QT
                    vf = vstage.tile([128, CV], F32, tag="vf")
                    nc.sync.dma_start(
                        vf[0:qn, :], vt[qc * 128 : qc * 128 + qn, :]
                    )
                    nc.vector.tensor_copy(
                        vt_all[0:qn, qc * CV : (qc + 1) * CV], vf[0:qn, :]
                    )

                # 1/denominator, broadcast to full width (per p-block)
                bcd_sb, bcd_raw = [], []
                for pb in range(2):
                    u = bmini.tile([1, PB], F32, tag=f"ud{pb}")
                    nc.scalar.copy(u[:], dns[pb][:])
                    bcp = bps.tile([128, PB], F32, tag="bcscr", name=f"bd{pb}")
                    nc.tensor.matmul(bcp[:], ones_row[:], u[:])
                    raw = bmini.tile([128, PB], F32, tag=f"dnraw{pb}")
                    nc.scalar.copy(raw[:], bcp[:])
                    inv = bmini.tile([128, PB], F32, tag=f"dninv{pb}")
                    nc.vector.reciprocal(inv[:], bcp[:])
                    bcd_sb.append(inv)
                    bcd_raw.append(raw)

                ssy = [
                    bps.tile([1, PB], F32, tag=f"ssy{pb}", name=f"ssy{pb}")
                    for pb in range(2)
                ]
                att_sb = {}
                for cb in range(NCV):
                    ys = [
                        yps.tile([128, PB], F32, tag=f"y{pb}", name=f"y{cb}_{pb}")
                        for pb in range(2)
                    ]
                    for qc in range(NQC):
                        lhsT = vt_all[:, qc * CV + cb * 128 : qc * CV + (cb + 1) * 128]
                        for pb in range(2):
                            nc.tensor.matmul(
                                ys[pb][:],
                                lhsT,
                                expT[:, qc * P + pb * PB : qc * P + pb * PB + PB],
                                start=(qc == 0),
                                stop=(qc == NQC - 1),
                            )
                    for pb in range(2):
                        att = batt.tile(
                            [128, PB], F32, tag=f"att{cb}_{pb}", name=f"att{cb}_{pb}"
                        )
                        nc.vector.tensor_mul(att[:], ys[pb][:], bcd_sb[pb][:])
                        att_sb[(cb, pb)] = att
                        nc.sync.dma_start(
                            att_o[cb * 128 : (cb + 1) * 128, pb * PB : (pb + 1) * PB],
                            att[:],
                        )
                        sqy = bsq.tile([128, PB], BF16, tag="sqy")
                        nc.scalar.square(sqy[:], ys[pb][:])
                        nc.tensor.matmul(
                            ssy[pb][:],
                            ones_col[:],
                            sqy[:],
                            start=(cb == 0),
                            stop=(cb == NCV - 1),
                        )

                for pb in range(2):
                    u = bmini.tile([1, PB], F32, tag=f"us{pb}")
                    nc.scalar.copy(u[:], ssy[pb][:])
                    bcp = bps.tile([128, PB], F32, tag="bcscr", name=f"bs{pb}")
                    nc.tensor.matmul(bcp[:], ones_row[:], u[:])
                    st = bmini.tile([128, PB], F32, tag=f"sts{pb}")
                    # sqrt(ssy/ATT_WT^2) = ||Y||/0.3; reciprocal -> 0.3/||Y||
                    nc.scalar.activation(
                        st[:], bcp[:], AF.Sqrt, scale=float(1.0 / (ATT_WT * ATT_WT))
                    )
                    sinv = bmini.tile([128, PB], F32, tag=f"sinv{pb}")
                    nc.vector.reciprocal(sinv[:], st[:])
                    # fq = fqn + att * (denom * 0.3/||Y||)
                    s2 = bmini.tile([128, PB], F32, tag=f"s2{pb}")
                    nc.vector.tensor_mul(s2[:], bcd_raw[pb][:], sinv[:])
                    for cb in range(NCV):
                        t = bout.tile([128, PB], F32, tag="t")
                        nc.vector.tensor_mul(t[:], att_sb[(cb, pb)][:], s2[:])
                        f_sb = bout.tile([128, PB], F32, tag="f")
                        nc.vector.tensor_add(
                            f_sb[:],
                            t[:],
                            fqn[:, cb * P + pb * PB : cb * P + pb * PB + PB],
                        )
                        nc.sync.dma_start(
                            fq_o[cb * 128 : (cb + 1) * 128, pb * PB : (pb + 1) * PB],
                            f_sb[:],
                        )
    _split_sync_waits(nc)
    return nc


def make_in_maps(fq_l3, fs_l3, fq_l4, fs_l4, f_q, f_s, w_red):
    wvec = np.asarray(
        [[TEMP * float(w_red[0]), TEMP * float(w_red[1])]], dtype=np.float32
    )
    q4f = np.asarray(fq_l4, np.float32).reshape(B, C4, HW)
    q3f = np.asarray(fq_l3, np.float32).reshape(B, C3, HW)
    s4f = np.asarray(fs_l4, np.float32).reshape(B, C4, HW)
    s3f = np.asarray(fs_l3, np.float32).reshape(B, C3, HW)
    vf = np.asarray(f_s, np.float32).reshape(B, CV, HW)
    fqf = np.asarray(f_q, np.float32).reshape(B, CV, HW)
    in_maps = []
    for k in range(NCORES):
        b, j = divmod(k, PSH)
        sl = slice(j * P, (j + 1) * P)
        in_maps.append(
            {
                "q4": np.ascontiguousarray(q4f[b][:, sl]),
                "q3": np.ascontiguousarray(q3f[b][:, sl]),
                "s4": np.ascontiguousarray(s4f[b]),
                "s3": np.ascontiguousarray(s3f[b]),
                "vt": np.ascontiguousarray(vf[b].T),
                "fq": np.ascontiguousarray(fqf[b][:, sl]),
                "wv": wvec,
            }
        )
    return in_maps


def gather_outputs(results):
    att = np.empty((B, CV, HW), np.float32)
    fqo = np.empty((B, CV, HW), np.float32)
    for k in range(NCORES):
        b, j = divmod(k, PSH)
        sl = slice(j * P, (j + 1) * P)
        att[b][:, sl] = results[k]["att_o"]
        fqo[b][:, sl] = results[k]["fq_o"]
    return (
        fqo.reshape(B, CV, H, W),
        att.reshape(B, CV, H, W),
    )


def kernel(fq_l3, fs_l3, fq_l4, fs_l4, f_q, f_s, w_red, trace=False):
    nc = build()
    in_maps = make_in_maps(fq_l3, fs_l3, fq_l4, fs_l4, f_q, f_s, w_red)
    res = run_bass_kernel_spmd(nc, in_maps, core_ids=list(range(NCORES)), trace=trace)
    out = gather_outputs(res.results)
    if trace:
        return out, res
    return out



# revision 11
# speedup vs baseline: 1.7660x; 1.7660x over previous
"""Trainium2 Bass kernel for nn_MMN_7361573945989 (MatchNet corr/attention).

Math (per batch b):
  qn_l = l2norm_c(fq_l); sn_l = l2norm_c(fs_l)           l in {4, 3}
  logits[p, q] = TEMP * (w0 * qn4.T@sn4 + w1 * qn3.T@sn3)[p, q]
  attn = softmax_q(logits)
  att_fq[c, p] = sum_q attn[p, q] * f_s[c, q]
  fq_out = l2norm_c(f_q) + l2norm_c(att_fq) * ATT_WT
  returns (fq_out, att_fq)

Sharding: 8 cores = 2 batches x 4 query-pixel shards of 900.

Per-core kernel (transposed orientation, logits live as [q, p] tiles):
  - all features arrive bf16 from the host; f_s pre-transposed to [hw, cv]
  - query side (P0): squares on DVE (bf16 4x), channel sums via ones-column
    matmuls into [1, PB] PSUM rows, broadcast via K=1 matmul, and the
    inverse norm (scaled by |TEMP*w_l|) comes from one activation:
    Exp(-0.5*Ln(ss) + ln|T*w_l|).  Sign of w_l folds into the query scaling.
  - support side (main loop, per 128-pixel chunk): squares on DVE, then 24
    single-column matmuls (lhsT = squared chunk, rhs = ones) contract the
    channel partition dim directly into per-layer [q, 1] PSUM columns; the
    inverse norms are per-partition [128,1] scalars -- no broadcasts.
  - logits accumulate per layer in separate PSUM groups (ps4 double-
    buffered across chunks, ps3 single), and the softmax exp is split as
    exp(l4 + l3) = Exp(ps4 * inv4[q]) * Exp(ps3 * inv3[q]) using the
    activation's per-partition scale operand; the product is one cheap
    bf16 DVE multiply into the persistent exp table.
  - only Exp/Ln/Square/Copy activations are used -> a single activation
    table ('natural_log_exp_and_others'), no table reloads.
  - phase B: two passes of 2 channel blocks; Y[c,p] accumulates exp @ f_s.T
    over chunks; softmax denominators via ones-matmuls in pass 0; the
    att_fq l2norm uses ||Y|| so the denominator cancels.
"""

import math
import sys
from contextlib import ExitStack

import numpy as np
import ml_dtypes

sys.path.insert(0, "/opt/trn_rl_repo")

import concourse.bass as bass  # noqa: E402
import concourse.tile as tile  # noqa: E402
from concourse import mybir  # noqa: E402
from concourse.bass_utils import run_bass_kernel_spmd  # noqa: E402

B, H, W = 2, 60, 60
HW = H * W  # 3600
C3, C4, CV = 1024, 2048, 512
TEMP = 20.0
ATT_WT = 0.3
NCORES = 8
PSH = 4  # query-pixel shards per batch
P = HW // PSH  # 900 query pixels per core
PB = P // 2  # 450, p-block (one PSUM bank of fp32)
NQC = (HW + 127) // 128  # 29 support-pixel chunks
QT = HW - (NQC - 1) * 128  # 16 rows in the tail chunk
NC4, NC3, NCV = C4 // 128, C3 // 128, CV // 128  # 16, 8, 4
NCI = NC4 + NC3  # 24 combined channel chunks

F32 = mybir.dt.float32
BF16 = mybir.dt.bfloat16
AF = mybir.ActivationFunctionType
MUL = mybir.AluOpType.mult

_MAX_WAITS_PER_INST = 1


def _patched_drain_and_barrier(self, tick_clock, wait_clock):
    """Tile's kernel-tail drain carries one sem wait per engine/queue; the
    walrus build used here accepts only one sync wait per CTRL instruction.
    Split the waits across extra sync-engine nops."""
    drain_inst = self.nc.sync.drain()
    wait_clock.add_sem_waits(
        drain_inst.ins, tile.ScopedClock({None: tick_clock.global_clock})
    )
    si = drain_inst.ins.sync_info
    if si is not None and len(si.on_wait) > _MAX_WAITS_PER_INST:
        waits = list(si.on_wait)
        drain_inst.ins.sync_info = mybir.SyncInfo(
            on_wait=waits[:_MAX_WAITS_PER_INST], on_update=list(si.on_update)
        )
        for i in range(_MAX_WAITS_PER_INST, len(waits), _MAX_WAITS_PER_INST):
            nop = self.nc.sync.nop()
            nop.ins.sync_info = mybir.SyncInfo(
                on_wait=waits[i : i + _MAX_WAITS_PER_INST], on_update=[]
            )
    self.nc.all_engine_barrier()
    assert self.sems is not None
    popped = self.nc._tile_sem_poison_stack.pop()
    assert popped is self._sem_poison
    self.nc.clear_and_free_semaphores(list(self.sems.allocated().values()))
    self.nc.all_engine_barrier()


tile.TileContext._drain_and_barrier = _patched_drain_and_barrier


def _split_sync_waits(nc, max_waits=_MAX_WAITS_PER_INST):
    """Walrus here accepts at most one sync wait per instruction; move excess
    waits onto same-engine nops inserted immediately before the instruction."""
    ctr = 0
    for f in nc.m.functions:
        for blk in f.blocks:
            insts = list(blk.instructions)
            out = []
            changed = False
            for inst in insts:
                si = inst.sync_info
                if si is not None and len(si.on_wait) > max_waits:
                    waits = list(si.on_wait)
                    for i0 in range(max_waits, len(waits), max_waits):
                        ctr += 1
                        nop = mybir.InstNoOp(
                            name=f"waitsplit-{ctr}",
                            engine=inst.engine,
                            bass_nofuse=True,
                            sync_info=mybir.SyncInfo(
                                on_wait=waits[i0 : i0 + max_waits], on_update=[]
                            ),
                        )
                        nc.register_instruction(nop, overwrite=True)
                        out.append(nop)
                    inst.sync_info = mybir.SyncInfo(
                        on_wait=waits[:max_waits], on_update=list(si.on_update)
                    )
                    changed = True
                out.append(inst)
            if changed:
                blk.instructions = out


def build():
    nc = bass.Bass()
    q4 = nc.dram_tensor("q4", [C4, P], BF16, kind="ExternalInput")
    q3 = nc.dram_tensor("q3", [C3, P], BF16, kind="ExternalInput")
    s4 = nc.dram_tensor("s4", [C4, HW], BF16, kind="ExternalInput")
    s3 = nc.dram_tensor("s3", [C3, HW], BF16, kind="ExternalInput")
    vt = nc.dram_tensor("vt", [HW, CV], BF16, kind="ExternalInput")  # f_s.T
    fq = nc.dram_tensor("fq", [CV, P], BF16, kind="ExternalInput")
    # wv = [[ln|T*w0|, ln|T*w1|]]
    wv = nc.dram_tensor("wv", [1, 2], F32, kind="ExternalInput")
    att_o = nc.dram_tensor("att_o", [CV, P], F32, kind="ExternalOutput")
    fq_o = nc.dram_tensor("fq_o", [CV, P], F32, kind="ExternalOutput")

    def load_blocks(dst, dst_cols, ci0, src, col0, ncols, n_ci, group=8):
        """Load `n_ci` row-blocks of 128 from DRAM `src` (cols [col0,col0+ncols))
        into SBUF tile `dst` whose free layout is (ci, dst_cols)."""
        srcr = src[:].rearrange("(ci c) x -> c ci x", c=128)
        dstr = dst[:].rearrange("c (ci x) -> c ci x", x=dst_cols)
        for g0 in range(0, n_ci, group):
            g = min(group, n_ci - g0)
            nc.sync.dma_start(
                dstr[:, ci0 + g0 : ci0 + g0 + g, 0:ncols],
                srcr[:, g0 : g0 + g, col0 : col0 + ncols],
            )

    with tile.TileContext(nc) as tc:
        with ExitStack() as octx:
            cpool = octx.enter_context(tc.tile_pool(name="const", bufs=1))
            ones_col = cpool.tile([128, 1], BF16)
            nc.gpsimd.memset(ones_col[:], 1.0)
            ones_row = cpool.tile([1, 128], F32)
            nc.gpsimd.memset(ones_row[:], 1.0)
            w_sb = cpool.tile([1, 2], F32)
            nc.sync.dma_start(w_sb[:], wv[:])
            w_col = cpool.tile([128, 2], F32)
            lnw_att = cpool.tile([128, 1], F32)
            nc.gpsimd.memset(lnw_att[:], float(math.log(ATT_WT)))

            pers = octx.enter_context(tc.tile_pool(name="pers", bufs=1))
            qns = pers.tile([128, NCI * P], BF16)  # scaled query feats (ci, p)
            fqn = pers.tile([128, NCV * P], BF16)  # normalized f_q (ci, p)
            expT = pers.tile([128, NQC * P], BF16)  # exp(logits) (qc; q, p)
            vt_all = pers.tile([128, NQC * CV], BF16)  # f_s.T chunks (qc; q, c)
            # zero the tail-chunk regions so K=128 matmuls over the tail are
            # exact (rows [0:QT] get real data later)
            nc.gpsimd.memset(expT[:, (NQC - 1) * P : NQC * P], 0.0)
            nc.gpsimd.memset(vt_all[:, (NQC - 1) * CV : NQC * CV], 0.0)

            # broadcast ln|T*w| across partitions once: [1,2] -> [128,2]
            with tc.tile_pool(name="wps", bufs=1, space="PSUM") as wps:
                w_ps = wps.tile([128, 2], F32)
                nc.tensor.matmul(w_ps[:], ones_row[:], w_sb[:])
                nc.scalar.copy(w_col[:], w_ps[:])

            # ---------------- P0: query-side normalization ----------------
            # dst tile, ci0, n_ci, src, w-col index (None => no |T*w|);
            # sign(w_l) is folded into the query arrays on the host
            layers = [
                (qns, 0, NC4, q4, 0),
                (qns, NC4, NC3, q3, 1),
                (fqn, 0, NCV, fq, None),
            ]
            with ExitStack() as pctx:
                sqp = pctx.enter_context(tc.tile_pool(name="p0sq", bufs=2))
                mini = pctx.enter_context(tc.tile_pool(name="p0mini", bufs=2))
                pps = pctx.enter_context(
                    tc.tile_pool(name="p0ps", bufs=1, space="PSUM")
                )
                for li, (dst, ci0, n_ci, src, wl) in enumerate(layers):
                    load_blocks(dst, P, ci0, src, 0, P, n_ci)
                    ss = [
                        pps.tile([1, PB], F32, tag=f"ss{li}_{pb}", name=f"ss{li}_{pb}")
                        for pb in range(2)
                    ]
                    for k in range(n_ci):
                        ci = ci0 + k
                        xg = dst[:, ci * P : (ci + 1) * P]
                        sq = sqp.tile([128, P], BF16, tag="sq", name=f"sqq{li}_{k}")
                        nc.vector.tensor_mul(sq[:], xg, xg)
                        for pb in range(2):
                            nc.tensor.matmul(
                                ss[pb][:],
                                ones_col[:],
                                sq[:, pb * PB : (pb + 1) * PB],
                                start=(k == 0),
                                stop=(k == n_ci - 1),
                            )
                    for pb in range(2):
                        u = mini.tile([1, PB], F32, tag="u", name=f"u{li}_{pb}")
                        nc.scalar.copy(u[:], ss[pb][:])
                        bc = pps.tile(
                            [128, PB], F32, tag="bc", name=f"bc{li}_{pb}", bufs=2
                        )
                        nc.tensor.matmul(bc[:], ones_row[:], u[:])
                        lnb = mini.tile(
                            [128, PB], F32, tag="lnb", name=f"lnb{li}_{pb}"
                        )
                        nc.scalar.activation(lnb[:], bc[:], AF.Ln)
                        invq = mini.tile(
                            [128, PB], BF16, tag="invq", name=f"invq{li}_{pb}"
                        )
                        # Exp(-0.5*ln(ss) + ln|T*w|) = |T*w| / sqrt(ss)
                        if wl is None:
                            nc.scalar.activation(
                                invq[:], lnb[:], AF.Exp, scale=-0.5
                            )
                        else:
                            nc.scalar.activation(
                                invq[:],
                                lnb[:],
                                AF.Exp,
                                scale=-0.5,
                                bias=w_col[:, wl : wl + 1],
                            )
                        for k in range(n_ci):
                            ci = ci0 + k
                            sl = slice(ci * P + pb * PB, ci * P + pb * PB + PB)
                            nc.vector.tensor_mul(dst[:, sl], dst[:, sl], invq[:])

            # ------------- main: support stream, logits, exp -------------
            with ExitStack() as mctx:
                snpool = mctx.enter_context(tc.tile_pool(name="sn", bufs=3))
                sqpool = mctx.enter_context(tc.tile_pool(name="msq", bufs=2))
                mmini = mctx.enter_context(tc.tile_pool(name="mmini", bufs=2))
                epool = mctx.enter_context(tc.tile_pool(name="etmp", bufs=2))
                mps = mctx.enter_context(
                    tc.tile_pool(name="mps", bufs=1, space="PSUM")
                )

                for qc in range(NQC):
                    qn = 128 if qc < NQC - 1 else QT
                    sn = snpool.tile([128, NCI * 128], BF16, tag="sn", name=f"sn{qc}")
                    load_blocks(sn, 128, 0, s4, qc * 128, qn, NC4)
                    load_blocks(sn, 128, NC4, s3, qc * 128, qn, NC3)
                    nc.sync.dma_start(
                        vt_all[0:qn, qc * CV : (qc + 1) * CV],
                        vt[qc * 128 : qc * 128 + qn, :],
                    )

                    sq = sqpool.tile([128, NCI * 128], BF16, tag="sq", name=f"sq{qc}")
                    nc.vector.tensor_mul(sq[:], sn[:], sn[:])

                    # logits: per-layer PSUM groups; lhsT reused across both
                    # p-blocks (2 matmuls per LDWEIGHTS)
                    ps4 = [
                        mps.tile(
                            [128, PB], F32, tag=f"ps4_{pb}", name=f"ps4_{pb}_{qc}",
                            bufs=2,
                        )
                        for pb in range(2)
                    ]
                    ps3 = [
                        mps.tile(
                            [128, PB], F32, tag=f"ps3_{pb}", name=f"ps3_{pb}_{qc}"
                        )
                        for pb in range(2)
                    ]
                    for ci in range(NCI):
                        lhsT = sn[:, ci * 128 : ci * 128 + qn]
                        pss = ps4 if ci < NC4 else ps3
                        for pb in range(2):
                            nc.tensor.matmul(
                                pss[pb][0:qn, :],
                                lhsT,
                                qns[:, ci * P + pb * PB : ci * P + pb * PB + PB],
                                start=(ci == 0 or ci == NC4),
                                stop=(ci == NC4 - 1 or ci == NCI - 1),
                            )
                    # support norms (after logits so the PE never waits on the
                    # fresh chunk's squares): contract the channel partition
                    # dim per chunk into per-layer [q, 1] PSUM columns
                    sscols = [
                        mps.tile([128, 1], F32, tag="ss4", name=f"ss4_{qc}"),
                        mps.tile([128, 1], F32, tag="ss3", name=f"ss3_{qc}"),
                    ]
                    for ci in range(NCI):
                        l = 0 if ci < NC4 else 1
                        nc.tensor.matmul(
                            sscols[l][0:qn, :],
                            sq[:, ci * 128 : ci * 128 + qn],
                            ones_col[:],
                            start=(ci == 0 or ci == NC4),
                            stop=(ci == NC4 - 1 or ci == NCI - 1),
                        )
                    invc = []
                    for l in range(2):
                        lns = mmini.tile(
                            [128, 1], F32, tag=f"lns{l}", name=f"lns{l}_{qc}"
                        )
                        nc.scalar.activation(
                            lns[0:qn, :], sscols[l][0:qn, :], AF.Ln
                        )
                        inv = mmini.tile(
                            [128, 1], F32, tag=f"inv{l}", name=f"inv{l}_{qc}"
                        )
                        nc.scalar.activation(
                            inv[0:qn, :], lns[0:qn, :], AF.Exp, scale=-0.5
                        )
                        invc.append(inv)
                    for pb in range(2):
                        e4 = epool.tile(
                            [128, PB], BF16, tag=f"e4_{pb}", name=f"e4_{pb}_{qc}"
                        )
                        nc.scalar.activation(
                            e4[0:qn, :],
                            ps4[pb][0:qn, :],
                            AF.Exp,
                            scale=invc[0][0:qn, :],
                        )
                        e3 = epool.tile(
                            [128, PB], BF16, tag=f"e3_{pb}", name=f"e3_{pb}_{qc}"
                        )
                        nc.scalar.activation(
                            e3[0:qn, :],
                            ps3[pb][0:qn, :],
                            AF.Exp,
                            scale=invc[1][0:qn, :],
                        )
                        nc.vector.tensor_mul(
                            expT[0:qn, qc * P + pb * PB : qc * P + pb * PB + PB],
                            e4[0:qn, :],
                            e3[0:qn, :],
                        )

            # ---------------- phase B: attention-weighted values ----------------
            with ExitStack() as bctx:
                bps = bctx.enter_context(
                    tc.tile_pool(name="bps", bufs=1, space="PSUM")
                )
                bmini = bctx.enter_context(tc.tile_pool(name="bmini", bufs=1))
                bsq = bctx.enter_context(tc.tile_pool(name="bsq", bufs=2))
                batt = bctx.enter_context(tc.tile_pool(name="batt", bufs=1))
                bout = bctx.enter_context(tc.tile_pool(name="bout", bufs=2))

                ssy = [
                    bps.tile([1, PB], F32, tag=f"ssy{pb}", name=f"ssy{pb}")
                    for pb in range(2)
                ]
                dns = [
                    bps.tile([1, PB], F32, tag=f"dn{pb}", name=f"dn{pb}")
                    for pb in range(2)
                ]
                att_sb = {}
                inv_dn, raw_dn = [], []
                for pss in range(2):
                    ys = {}
                    for cbk in range(2):
                        cb = 2 * pss + cbk
                        for pb in range(2):
                            ys[(cb, pb)] = bps.tile(
                                [128, PB], F32, tag=f"y{cbk}_{pb}",
                                name=f"y{cb}_{pb}",
                            )
                    for qc in range(NQC):
                        for cbk in range(2):
                            cb = 2 * pss + cbk
                            lhsT = vt_all[
                                :, qc * CV + cb * 128 : qc * CV + (cb + 1) * 128
                            ]
                            for pb in range(2):
                                nc.tensor.matmul(
                                    ys[(cb, pb)][:],
                                    lhsT,
                                    expT[:, qc * P + pb * PB : qc * P + pb * PB + PB],
                                    start=(qc == 0),
                                    stop=(qc == NQC - 1),
                                )
                        if pss == 0:
                            for pb in range(2):
                                nc.tensor.matmul(
                                    dns[pb][:],
                                    ones_col[:],
                                    expT[:, qc * P + pb * PB : qc * P + pb * PB + PB],
                                    start=(qc == 0),
                                    stop=(qc == NQC - 1),
                                )
                    if pss == 0:
                        # 1/denominator, broadcast to full width (per p-block)
                        for pb in range(2):
                            u = bmini.tile([1, PB], F32, tag=f"ud{pb}", name=f"ud{pb}")
                            nc.scalar.copy(u[:], dns[pb][:])
                            bcp = bps.tile(
                                [128, PB], F32, tag=f"dn{pb}", name=f"bd{pb}"
                            )
                            nc.tensor.matmul(bcp[:], ones_row[:], u[:])
                            raw = bmini.tile(
                                [128, PB], F32, tag=f"dnraw{pb}", name=f"dnraw{pb}"
                            )
                            nc.scalar.copy(raw[:], bcp[:])
                            inv = bmini.tile(
                                [128, PB], F32, tag=f"dninv{pb}", name=f"dninv{pb}"
                            )
                            nc.vector.reciprocal(inv[:], bcp[:])
                            inv_dn.append(inv)
                            raw_dn.append(raw)
                    for cbk in range(2):
                        cb = 2 * pss + cbk
                        for pb in range(2):
                            att = batt.tile(
                                [128, PB], F32, tag=f"att{cb}_{pb}",
                                name=f"att{cb}_{pb}",
                            )
                            nc.vector.tensor_mul(
                                att[:], ys[(cb, pb)][:], inv_dn[pb][:]
                            )
                            att_sb[(cb, pb)] = att
                            nc.sync.dma_start(
                                att_o[
                                    cb * 128 : (cb + 1) * 128,
                                    pb * PB : (pb + 1) * PB,
                                ],
                                att[:],
                            )
                            sqy = bsq.tile([128, PB], BF16, tag="sqy")
                            nc.scalar.square(sqy[:], ys[(cb, pb)][:])
                            nc.tensor.matmul(
                                ssy[pb][:],
                                ones_col[:],
                                sqy[:],
                                start=(cb == 0),
                                stop=(cb == NCV - 1),
                            )

                for pb in range(2):
                    u = bmini.tile([1, PB], F32, tag=f"us{pb}", name=f"us{pb}")
                    nc.scalar.copy(u[:], ssy[pb][:])
                    bcp = bps.tile([128, PB], F32, tag=f"ssy{pb}", name=f"bs{pb}")
                    nc.tensor.matmul(bcp[:], ones_row[:], u[:])
                    lnb = bmini.tile([128, PB], F32, tag=f"lnbs{pb}", name=f"lnbs{pb}")
                    nc.scalar.activation(lnb[:], bcp[:], AF.Ln)
                    # Exp(-0.5*ln(ssy) + ln(0.3)) = 0.3/||Y||
                    sinv = bmini.tile([128, PB], F32, tag=f"sinv{pb}", name=f"sinv{pb}")
                    nc.scalar.activation(
                        sinv[:], lnb[:], AF.Exp, scale=-0.5, bias=lnw_att[:]
                    )
                    # fq = fqn + att * (denom * 0.3/||Y||)
                    s2 = bmini.tile([128, PB], F32, tag=f"s2{pb}", name=f"s2{pb}")
                    nc.vector.tensor_mul(s2[:], raw_dn[pb][:], sinv[:])
                    for cb in range(NCV):
                        t = bout.tile([128, PB], F32, tag="t")
                        nc.vector.tensor_mul(t[:], att_sb[(cb, pb)][:], s2[:])
                        f_sb = bout.tile([128, PB], F32, tag="f")
                        nc.vector.tensor_add(
                            f_sb[:],
                            t[:],
                            fqn[:, cb * P + pb * PB : cb * P + pb * PB + PB],
                        )
                        nc.sync.dma_start(
                            fq_o[cb * 128 : (cb + 1) * 128, pb * PB : (pb + 1) * PB],
                            f_sb[:],
                        )
    _split_sync_waits(nc)
    return nc


def make_in_maps(fq_l3, fs_l3, fq_l4, fs_l4, f_q, f_s, w_red):
    bf = ml_dtypes.bfloat16
    wr = np.asarray(w_red, np.float32)
    wvec = np.log(np.abs(TEMP * wr)).reshape(1, 2)
    # fold sign(w_l) into the query features; |T*w_l| rides in wvec
    q4f = float(np.sign(wr[0])) * np.asarray(fq_l4, np.float32).reshape(B, C4, HW)
    q3f = float(np.sign(wr[1])) * np.asarray(fq_l3, np.float32).reshape(B, C3, HW)
    s4f = np.asarray(fs_l4, np.float32).reshape(B, C4, HW)
    s3f = np.asarray(fs_l3, np.float32).reshape(B, C3, HW)
    vf = np.asarray(f_s, np.float32).reshape(B, CV, HW)
    fqf = np.asarray(f_q, np.float32).reshape(B, CV, HW)
    s4b = [np.ascontiguousarray(s4f[b]).astype(bf) for b in range(B)]
    s3b = [np.ascontiguousarray(s3f[b]).astype(bf) for b in range(B)]
    vtb = [np.ascontiguousarray(vf[b].T).astype(bf) for b in range(B)]
    in_maps = []
    for k in range(NCORES):
        b, j = divmod(k, PSH)
        sl = slice(j * P, (j + 1) * P)
        in_maps.append(
            {
                "q4": np.ascontiguousarray(q4f[b][:, sl]).astype(bf),
                "q3": np.ascontiguousarray(q3f[b][:, sl]).astype(bf),
                "s4": s4b[b],
                "s3": s3b[b],
                "vt": vtb[b],
                "fq": np.ascontiguousarray(fqf[b][:, sl]).astype(bf),
                "wv": np.ascontiguousarray(wvec, np.float32),
            }
        )
    return in_maps


def gather_outputs(results):
    att = np.empty((B, CV, HW), np.float32)
    fqo = np.empty((B, CV, HW), np.float32)
    for k in range(NCORES):
        b, j = divmod(k, PSH)
        sl = slice(j * P, (j + 1) * P)
        att[b][:, sl] = results[k]["att_o"]
        fqo[b][:, sl] = results[k]["fq_o"]
    return (
        fqo.reshape(B, CV, H, W),
        att.reshape(B, CV, H, W),
    )


def kernel(fq_l3, fs_l3, fq_l4, fs_l4, f_q, f_s, w_red, trace=False):
    nc = build()
    in_maps = make_in_maps(fq_l3, fs_l3, fq_l4, fs_l4, f_q, f_s, w_red)
    res = run_bass_kernel_spmd(nc, in_maps, core_ids=list(range(NCORES)), trace=trace)
    out = gather_outputs(res.results)
    if trace:
        return out, res
    return out


# revision 20
# speedup vs baseline: 2.1640x; 1.2253x over previous
"""Trainium2 Bass kernel for nn_MMN_7361573945989 (MatchNet corr/attention).

Math (per batch b):
  qn_l = l2norm_c(fq_l); sn_l = l2norm_c(fs_l)           l in {4, 3}
  logits[p, q] = TEMP * (w0 * qn4.T@sn4 + w1 * qn3.T@sn3)[p, q]
  attn = softmax_q(logits)
  att_fq[c, p] = sum_q attn[p, q] * f_s[c, q]
  fq_out = l2norm_c(f_q) + l2norm_c(att_fq) * ATT_WT
  returns (fq_out, att_fq)

Sharding: 8 cores = 2 batches x 4 query-pixel shards of 900.

Per-core kernel (transposed orientation, logits live as [q, p] tiles):
  - all features arrive bf16 from the host; f_s pre-transposed to [hw, cv]
  - query side (P0): squares on DVE (bf16 4x), channel sums via ones-column
    matmuls into [1, PB] PSUM rows, broadcast via K=1 matmul, and the
    inverse norm (scaled by |TEMP*w_l|) comes from one activation:
    Exp(-0.5*Ln(ss) + ln|T*w_l|).  Sign of w_l folds into the query scaling.
  - support side (main loop, per 128-pixel chunk): squares on DVE, then 24
    single-column matmuls (lhsT = squared chunk, rhs = ones) contract the
    channel partition dim directly into per-layer [q, 1] PSUM columns; the
    inverse norms are per-partition [128,1] scalars -- no broadcasts.
  - logits accumulate per layer in separate PSUM groups (ps4 double-
    buffered across chunks, ps3 single), and the softmax exp is split as
    exp(l4 + l3) = Exp(ps4 * inv4[q]) * Exp(ps3 * inv3[q]) using the
    activation's per-partition scale operand; the product is one cheap
    bf16 DVE multiply into the persistent exp table.
  - only Exp/Ln/Square/Copy activations are used -> a single activation
    table ('natural_log_exp_and_others'), no table reloads.
  - phase B: two passes of 2 channel blocks; Y[c,p] accumulates exp @ f_s.T
    over chunks; softmax denominators via ones-matmuls in pass 0; the
    att_fq l2norm uses ||Y|| so the denominator cancels.
"""

import math
import sys
from contextlib import ExitStack

import numpy as np
import ml_dtypes

sys.path.insert(0, "/opt/trn_rl_repo")

import concourse.bass as bass  # noqa: E402
import concourse.tile as tile  # noqa: E402
from concourse import mybir  # noqa: E402
from concourse.bass_utils import run_bass_kernel_spmd  # noqa: E402

B, H, W = 2, 60, 60
HW = H * W  # 3600
C3, C4, CV = 1024, 2048, 512
TEMP = 20.0
ATT_WT = 0.3
NCORES = 8
PSH = 4  # query-pixel shards per batch
P = HW // PSH  # 900 query pixels per core
PB = P // 2  # 450, p-block (one PSUM bank of fp32)
NQC = (HW + 127) // 128  # 29 support-pixel chunks
QT = HW - (NQC - 1) * 128  # 16 rows in the tail chunk
NC4, NC3, NCV = C4 // 128, C3 // 128, CV // 128  # 16, 8, 4
NCI = NC4 + NC3  # 24 combined channel chunks

F32 = mybir.dt.float32
BF16 = mybir.dt.bfloat16
F8 = mybir.dt.float8e4
AF = mybir.ActivationFunctionType
MUL = mybir.AluOpType.mult
DR = mybir.MatmulPerfMode.DoubleRow

_MAX_WAITS_PER_INST = 1


def _patched_drain_and_barrier(self, tick_clock, wait_clock):
    """Tile's kernel-tail drain carries one sem wait per engine/queue; the
    walrus build used here accepts only one sync wait per CTRL instruction.
    Split the waits across extra sync-engine nops."""
    drain_inst = self.nc.sync.drain()
    wait_clock.add_sem_waits(
        drain_inst.ins, tile.ScopedClock({None: tick_clock.global_clock})
    )
    si = drain_inst.ins.sync_info
    if si is not None and len(si.on_wait) > _MAX_WAITS_PER_INST:
        waits = list(si.on_wait)
        drain_inst.ins.sync_info = mybir.SyncInfo(
            on_wait=waits[:_MAX_WAITS_PER_INST], on_update=list(si.on_update)
        )
        for i in range(_MAX_WAITS_PER_INST, len(waits), _MAX_WAITS_PER_INST):
            nop = self.nc.sync.nop()
            nop.ins.sync_info = mybir.SyncInfo(
                on_wait=waits[i : i + _MAX_WAITS_PER_INST], on_update=[]
            )
    self.nc.all_engine_barrier()
    assert self.sems is not None
    popped = self.nc._tile_sem_poison_stack.pop()
    assert popped is self._sem_poison
    self.nc.clear_and_free_semaphores(list(self.sems.allocated().values()))
    self.nc.all_engine_barrier()


tile.TileContext._drain_and_barrier = _patched_drain_and_barrier


def _split_sync_waits(nc, max_waits=_MAX_WAITS_PER_INST):
    """Walrus here accepts at most one sync wait per instruction; move excess
    waits onto same-engine nops inserted immediately before the instruction."""
    ctr = 0
    for f in nc.m.functions:
        for blk in f.blocks:
            insts = list(blk.instructions)
            out = []
            changed = False
            for inst in insts:
                si = inst.sync_info
                if si is not None and len(si.on_wait) > max_waits:
                    waits = list(si.on_wait)
                    for i0 in range(max_waits, len(waits), max_waits):
                        ctr += 1
                        nop = mybir.InstNoOp(
                            name=f"waitsplit-{ctr}",
                            engine=inst.engine,
                            bass_nofuse=True,
                            sync_info=mybir.SyncInfo(
                                on_wait=waits[i0 : i0 + max_waits], on_update=[]
                            ),
                        )
                        nc.register_instruction(nop, overwrite=True)
                        out.append(nop)
                    inst.sync_info = mybir.SyncInfo(
                        on_wait=waits[:max_waits], on_update=list(si.on_update)
                    )
                    changed = True
                out.append(inst)
            if changed:
                blk.instructions = out


def build():
    nc = bass.Bass()
    q4 = nc.dram_tensor("q4", [C4, P], BF16, kind="ExternalInput")
    q3 = nc.dram_tensor("q3", [C3, P], BF16, kind="ExternalInput")
    s4 = nc.dram_tensor("s4", [C4, HW], F8, kind="ExternalInput")
    s4b = nc.dram_tensor("s4b", [C4, HW], BF16, kind="ExternalInput")
    s3 = nc.dram_tensor("s3", [C3, HW], BF16, kind="ExternalInput")
    vt = nc.dram_tensor("vt", [HW, CV], BF16, kind="ExternalInput")  # f_s.T
    fq = nc.dram_tensor("fq", [CV, P], BF16, kind="ExternalInput")
    # wv = [[ln|T*w0|, ln|T*w1|]]
    wv = nc.dram_tensor("wv", [1, 2], F32, kind="ExternalInput")
    att_o = nc.dram_tensor("att_o", [CV, P], F32, kind="ExternalOutput")
    fq_o = nc.dram_tensor("fq_o", [CV, P], F32, kind="ExternalOutput")

    def load_blocks(dst, dst_cols, ci0, src, col0, ncols, n_ci, group=8, eng=None):
        """Load `n_ci` row-blocks of 128 from DRAM `src` (cols [col0,col0+ncols))
        into SBUF tile `dst` whose free layout is (ci, dst_cols)."""
        eng = eng or nc.sync
        srcr = src[:].rearrange("(ci c) x -> c ci x", c=128)
        dstr = dst[:].rearrange("c (ci x) -> c ci x", x=dst_cols)
        for g0 in range(0, n_ci, group):
            g = min(group, n_ci - g0)
            eng.dma_start(
                dstr[:, ci0 + g0 : ci0 + g0 + g, 0:ncols],
                srcr[:, g0 : g0 + g, col0 : col0 + ncols],
            )

    with tile.TileContext(nc) as tc:
        with ExitStack() as octx:
            cpool = octx.enter_context(tc.tile_pool(name="const", bufs=1))
            ones_col = cpool.tile([128, 1], BF16)
            nc.gpsimd.memset(ones_col[:], 1.0)
            ones_row = cpool.tile([1, 128], F32)
            nc.gpsimd.memset(ones_row[:], 1.0)
            w_sb = cpool.tile([1, 2], F32)
            nc.sync.dma_start(w_sb[:], wv[:])
            w_col = cpool.tile([128, 2], F32)
            lnw_att = cpool.tile([128, 1], F32)
            nc.gpsimd.memset(lnw_att[:], float(math.log(ATT_WT)))

            pers = octx.enter_context(tc.tile_pool(name="pers", bufs=1))
            qns4 = pers.tile([128, NC4 * P], F8)  # scaled query l4 (ci, p)
            qns3 = pers.tile([128, NC3 * P], BF16)  # scaled query l3 (ci, p)
            fqn = pers.tile([128, NCV * P], BF16)  # normalized f_q (ci, p)
            expT = pers.tile([128, NQC * P], BF16)  # exp(logits) (qc; q, p)
            vt_all = pers.tile([128, NQC * CV], BF16)  # f_s.T chunks (qc; q, c)
            # zero the tail-chunk regions so K=128 matmuls over the tail are
            # exact (rows [0:QT] get real data later)
            nc.gpsimd.memset(expT[:, (NQC - 1) * P : NQC * P], 0.0)
            nc.gpsimd.memset(vt_all[:, (NQC - 1) * CV : NQC * CV], 0.0)

            # broadcast ln|T*w| across partitions once: [1,2] -> [128,2]
            with tc.tile_pool(name="wps", bufs=1, space="PSUM") as wps:
                w_ps = wps.tile([128, 2], F32)
                nc.tensor.matmul(w_ps[:], ones_row[:], w_sb[:])
                nc.scalar.copy(w_col[:], w_ps[:])

            # ---------------- P0: query-side normalization ----------------
            # src tile (raw bf16), dst tile, n_ci, dram src, w-col index
            # (None => no |T*w|); sign(w_l) is folded on the host
            with ExitStack() as pctx:
                sqp = pctx.enter_context(tc.tile_pool(name="p0sq", bufs=2))
                mini = pctx.enter_context(tc.tile_pool(name="p0mini", bufs=2))
                stg = pctx.enter_context(tc.tile_pool(name="p0stg", bufs=1))
                pps = pctx.enter_context(
                    tc.tile_pool(name="p0ps", bufs=1, space="PSUM")
                )
                q4s = stg.tile([128, NC4 * P], BF16)  # raw q4 staging
                layers = [
                    (q4s, qns4, NC4, q4, 0),
                    (qns3, qns3, NC3, q3, 1),
                    (fqn, fqn, NCV, fq, None),
                ]
                for li, (raw, dst, n_ci, src, wl) in enumerate(layers):
                    load_blocks(raw, P, 0, src, 0, P, n_ci)
                    ss = [
                        pps.tile([1, PB], F32, tag=f"ss{li}_{pb}", name=f"ss{li}_{pb}")
                        for pb in range(2)
                    ]
                    for k in range(n_ci):
                        xg = raw[:, k * P : (k + 1) * P]
                        sq = sqp.tile([128, P], BF16, tag="sq", name=f"sqq{li}_{k}")
                        nc.vector.tensor_mul(sq[:], xg, xg)
                        for pb in range(2):
                            nc.tensor.matmul(
                                ss[pb][:],
                                ones_col[:],
                                sq[:, pb * PB : (pb + 1) * PB],
                                start=(k == 0),
                                stop=(k == n_ci - 1),
                            )
                    for pb in range(2):
                        u = mini.tile([1, PB], F32, tag="u", name=f"u{li}_{pb}")
                        nc.scalar.copy(u[:], ss[pb][:])
                        bc = pps.tile(
                            [128, PB], F32, tag="bc", name=f"bc{li}_{pb}", bufs=2
                        )
                        nc.tensor.matmul(bc[:], ones_row[:], u[:])
                        lnb = mini.tile(
                            [128, PB], F32, tag="lnb", name=f"lnb{li}_{pb}"
                        )
                        nc.scalar.activation(lnb[:], bc[:], AF.Ln)
                        invq = mini.tile(
                            [128, PB], BF16, tag="invq", name=f"invq{li}_{pb}"
                        )
                        # Exp(-0.5*ln(ss) + ln|T*w|) = |T*w| / sqrt(ss)
                        if wl is None:
                            nc.scalar.activation(
                                invq[:], lnb[:], AF.Exp, scale=-0.5
                            )
                        else:
                            nc.scalar.activation(
                                invq[:],
                                lnb[:],
                                AF.Exp,
                                scale=-0.5,
                                bias=w_col[:, wl : wl + 1],
                            )
                        for k in range(n_ci):
                            sl = slice(k * P + pb * PB, k * P + pb * PB + PB)
                            nc.vector.tensor_mul(
                                dst[:, sl], raw[:, sl], invq[:]
                            )

            # ------------- main: support stream, logits, exp -------------
            with ExitStack() as mctx:
                snpool = mctx.enter_context(tc.tile_pool(name="sn", bufs=3))
                sqpool = mctx.enter_context(tc.tile_pool(name="msq", bufs=2))
                mmini = mctx.enter_context(tc.tile_pool(name="mmini", bufs=2))
                epool = mctx.enter_context(tc.tile_pool(name="etmp", bufs=2))
                mps = mctx.enter_context(
                    tc.tile_pool(name="mps", bufs=1, space="PSUM")
                )

                qns4r = qns4[:].rearrange("c (ci p) -> c ci p", p=P)
                for qc in range(NQC):
                    qn = 128 if qc < NQC - 1 else QT
                    sn4 = snpool.tile(
                        [128, NC4 * 128], F8, tag="sn4", name=f"sn4_{qc}"
                    )
                    sn3 = snpool.tile(
                        [128, NC3 * 128], BF16, tag="sn3", name=f"sn3_{qc}"
                    )
                    load_blocks(sn4, 128, 0, s4, qc * 128, qn, NC4, eng=nc.gpsimd)
                    load_blocks(sn3, 128, 0, s3, qc * 128, qn, NC3, eng=nc.gpsimd)
                    nc.gpsimd.dma_start(
                        vt_all[0:qn, qc * CV : (qc + 1) * CV],
                        vt[qc * 128 : qc * 128 + qn, :],
                    )

                    # squares (for support norms) from the fp8/bf16 chunks
                    sq = sqpool.tile([128, NCI * 128], BF16, tag="sq", name=f"sq{qc}")
                    nc.vector.tensor_mul(sq[:, 0 : NC4 * 128], sn4[:], sn4[:])
                    nc.vector.tensor_mul(sq[:, NC4 * 128 :], sn3[:], sn3[:])

                    # logits: per-layer PSUM groups; layer 4 runs fp8
                    # DoubleRow (K=256 per instruction at 2x rate)
                    ps4 = [
                        mps.tile(
                            [128, PB], F32, tag=f"ps4_{pb}", name=f"ps4_{pb}_{qc}",
                            bufs=2,
                        )
                        for pb in range(2)
                    ]
                    ps3 = [
                        mps.tile(
                            [128, PB], F32, tag=f"ps3_{pb}", name=f"ps3_{pb}_{qc}"
                        )
                        for pb in range(2)
                    ]
                    sn4r = sn4[:].rearrange("c (ci q) -> c ci q", q=128)
                    for i in range(NC4 // 2):
                        lhsT = sn4r[:, 2 * i : 2 * i + 2, 0:qn]
                        for pb in range(2):
                            nc.tensor.matmul(
                                ps4[pb][0:qn, :],
                                lhsT,
                                qns4r[
                                    :, 2 * i : 2 * i + 2,
                                    pb * PB : pb * PB + PB,
                                ],
                                start=(i == 0),
                                stop=(i == NC4 // 2 - 1),
                                perf_mode=DR,
                            )
                    for k in range(NC3):
                        lhsT = sn3[:, k * 128 : k * 128 + qn]
                        for pb in range(2):
                            nc.tensor.matmul(
                                ps3[pb][0:qn, :],
                                lhsT,
                                qns3[:, k * P + pb * PB : k * P + pb * PB + PB],
                                start=(k == 0),
                                stop=(k == NC3 - 1),
                            )
                    # support norms (after logits so the PE never waits on the
                    # fresh chunk's squares): contract the channel partition
                    # dim per chunk into per-layer [q, 1] PSUM columns
                    sscols = [
                        mps.tile([128, 1], F32, tag="ss4", name=f"ss4_{qc}"),
                        mps.tile([128, 1], F32, tag="ss3", name=f"ss3_{qc}"),
                    ]
                    for ci in range(NCI):
                        l = 0 if ci < NC4 else 1
                        nc.tensor.matmul(
                            sscols[l][0:qn, :],
                            sq[:, ci * 128 : ci * 128 + qn],
                            ones_col[:],
                            start=(ci == 0 or ci == NC4),
                            stop=(ci == NC4 - 1 or ci == NCI - 1),
                        )
                    invc = []
                    for l in range(2):
                        lns = mmini.tile(
                            [128, 1], F32, tag=f"lns{l}", name=f"lns{l}_{qc}"
                        )
                        nc.scalar.activation(
                            lns[0:qn, :], sscols[l][0:qn, :], AF.Ln
                        )
                        inv = mmini.tile(
                            [128, 1], F32, tag=f"inv{l}", name=f"inv{l}_{qc}"
                        )
                        nc.scalar.activation(
                            inv[0:qn, :], lns[0:qn, :], AF.Exp, scale=-0.5
                        )
                        invc.append(inv)
                    for pb in range(2):
                        e4 = epool.tile(
                            [128, PB], BF16, tag=f"e4_{pb}", name=f"e4_{pb}_{qc}"
                        )
                        nc.scalar.activation(
                            e4[0:qn, :],
                            ps4[pb][0:qn, :],
                            AF.Exp,
                            scale=invc[0][0:qn, :],
                        )
                        e3 = epool.tile(
                            [128, PB], BF16, tag=f"e3_{pb}", name=f"e3_{pb}_{qc}"
                        )
                        nc.scalar.activation(
                            e3[0:qn, :],
                            ps3[pb][0:qn, :],
                            AF.Exp,
                            scale=invc[1][0:qn, :],
                        )
                        nc.vector.tensor_mul(
                            expT[0:qn, qc * P + pb * PB : qc * P + pb * PB + PB],
                            e4[0:qn, :],
                            e3[0:qn, :],
                        )

            # ---------------- phase B: attention-weighted values ----------------
            with ExitStack() as bctx:
                bps = bctx.enter_context(
                    tc.tile_pool(name="bps", bufs=1, space="PSUM")
                )
                bmini = bctx.enter_context(tc.tile_pool(name="bmini", bufs=1))
                bsq = bctx.enter_context(tc.tile_pool(name="bsq", bufs=2))
                batt = bctx.enter_context(tc.tile_pool(name="batt", bufs=1))
                bout = bctx.enter_context(tc.tile_pool(name="bout", bufs=2))

                ssy = [
                    bps.tile([1, PB], F32, tag=f"ssy{pb}", name=f"ssy{pb}")
                    for pb in range(2)
                ]
                dns = [
                    bps.tile([1, PB], F32, tag=f"dn{pb}", name=f"dn{pb}")
                    for pb in range(2)
                ]
                att_sb = {}
                inv_dn, raw_dn = [], []
                # denominators first: their matmuls are cheap and stopping the
                # accumulation early lets the 1/dn chain overlap the Y matmuls
                for qc in range(NQC):
                    for pb in range(2):
                        nc.tensor.matmul(
                            dns[pb][:],
                            ones_col[:],
                            expT[:, qc * P + pb * PB : qc * P + pb * PB + PB],
                            start=(qc == 0),
                            stop=(qc == NQC - 1),
                        )
                for pb in range(2):
                    u = bmini.tile([1, PB], F32, tag=f"ud{pb}", name=f"ud{pb}")
                    nc.scalar.copy(u[:], dns[pb][:])
                    bcp = bps.tile([128, PB], F32, tag=f"dn{pb}", name=f"bd{pb}")
                    nc.tensor.matmul(bcp[:], ones_row[:], u[:])
                    raw = bmini.tile(
                        [128, PB], F32, tag=f"dnraw{pb}", name=f"dnraw{pb}"
                    )
                    nc.scalar.copy(raw[:], bcp[:])
                    inv = bmini.tile(
                        [128, PB], F32, tag=f"dninv{pb}", name=f"dninv{pb}"
                    )
                    nc.vector.reciprocal(inv[:], bcp[:])
                    inv_dn.append(inv)
                    raw_dn.append(raw)
                for pss in range(2):
                    ys = {}
                    for cbk in range(2):
                        cb = 2 * pss + cbk
                        for pb in range(2):
                            ys[(cb, pb)] = bps.tile(
                                [128, PB], F32, tag=f"y{cbk}_{pb}",
                                name=f"y{cb}_{pb}",
                            )
                    for qc in range(NQC):
                        for cbk in range(2):
                            cb = 2 * pss + cbk
                            lhsT = vt_all[
                                :, qc * CV + cb * 128 : qc * CV + (cb + 1) * 128
                            ]
                            for pb in range(2):
                                nc.tensor.matmul(
                                    ys[(cb, pb)][:],
                                    lhsT,
                                    expT[:, qc * P + pb * PB : qc * P + pb * PB + PB],
                                    start=(qc == 0),
                                    stop=(qc == NQC - 1),
                                )
                    for cbk in range(2):
                        cb = 2 * pss + cbk
                        for pb in range(2):
                            att = batt.tile(
                                [128, PB], F32, tag=f"att{cb}_{pb}",
                                name=f"att{cb}_{pb}",
                            )
                            nc.vector.tensor_mul(
                                att[:], ys[(cb, pb)][:], inv_dn[pb][:]
                            )
                            att_sb[(cb, pb)] = att
                            nc.sync.dma_start(
                                att_o[
                                    cb * 128 : (cb + 1) * 128,
                                    pb * PB : (pb + 1) * PB,
                                ],
                                att[:],
                            )
                            sqy = bsq.tile([128, PB], BF16, tag="sqy")
                            nc.scalar.square(sqy[:], ys[(cb, pb)][:])
                            nc.tensor.matmul(
                                ssy[pb][:],
                                ones_col[:],
                                sqy[:],
                                start=(cb == 0),
                                stop=(cb == NCV - 1),
                            )

                for pb in range(2):
                    u = bmini.tile([1, PB], F32, tag=f"us{pb}", name=f"us{pb}")
                    nc.scalar.copy(u[:], ssy[pb][:])
                    bcp = bps.tile([128, PB], F32, tag=f"ssy{pb}", name=f"bs{pb}")
                    nc.tensor.matmul(bcp[:], ones_row[:], u[:])
                    lnb = bmini.tile([128, PB], F32, tag=f"lnbs{pb}", name=f"lnbs{pb}")
                    nc.scalar.activation(lnb[:], bcp[:], AF.Ln)
                    # Exp(-0.5*ln(ssy) + ln(0.3)) = 0.3/||Y||
                    sinv = bmini.tile([128, PB], F32, tag=f"sinv{pb}", name=f"sinv{pb}")
                    nc.scalar.activation(
                        sinv[:], lnb[:], AF.Exp, scale=-0.5, bias=lnw_att[:]
                    )
                    # fq = fqn + att * (denom * 0.3/||Y||)
                    s2 = bmini.tile([128, PB], F32, tag=f"s2{pb}", name=f"s2{pb}")
                    nc.vector.tensor_mul(s2[:], raw_dn[pb][:], sinv[:])
                    for cb in range(NCV):
                        t = bout.tile([128, PB], F32, tag="t")
                        nc.vector.tensor_mul(t[:], att_sb[(cb, pb)][:], s2[:])
                        f_sb = bout.tile([128, PB], F32, tag="f")
                        nc.vector.tensor_add(
                            f_sb[:],
                            t[:],
                            fqn[:, cb * P + pb * PB : cb * P + pb * PB + PB],
                        )
                        nc.sync.dma_start(
                            fq_o[cb * 128 : (cb + 1) * 128, pb * PB : (pb + 1) * PB],
                            f_sb[:],
                        )
    _split_sync_waits(nc)
    return nc


def make_in_maps(fq_l3, fs_l3, fq_l4, fs_l4, f_q, f_s, w_red):
    bf = ml_dtypes.bfloat16
    wr = np.asarray(w_red, np.float32)
    wvec = np.log(np.abs(TEMP * wr)).reshape(1, 2)
    # fold sign(w_l) into the query features; |T*w_l| rides in wvec
    q4f = float(np.sign(wr[0])) * np.asarray(fq_l4, np.float32).reshape(B, C4, HW)
    q3f = float(np.sign(wr[1])) * np.asarray(fq_l3, np.float32).reshape(B, C3, HW)
    s4f = np.asarray(fs_l4, np.float32).reshape(B, C4, HW)
    s3f = np.asarray(fs_l3, np.float32).reshape(B, C3, HW)
    vf = np.asarray(f_s, np.float32).reshape(B, CV, HW)
    fqf = np.asarray(f_q, np.float32).reshape(B, CV, HW)
    f8 = ml_dtypes.float8_e4m3
    s4b = [np.ascontiguousarray(s4f[b]).astype(bf) for b in range(B)]
    s48 = [x.astype(f8) for x in s4b]
    s3b = [np.ascontiguousarray(s3f[b]).astype(bf) for b in range(B)]
    vtb = [np.ascontiguousarray(vf[b].T).astype(bf) for b in range(B)]
    in_maps = []
    for k in range(NCORES):
        b, j = divmod(k, PSH)
        sl = slice(j * P, (j + 1) * P)
        in_maps.append(
            {
                "q4": np.ascontiguousarray(q4f[b][:, sl]).astype(bf),
                "q3": np.ascontiguousarray(q3f[b][:, sl]).astype(bf),
                "s4": s48[b],
                "s4b": s4b[b],
                "s3": s3b[b],
                "vt": vtb[b],
                "fq": np.ascontiguousarray(fqf[b][:, sl]).astype(bf),
                "wv": np.ascontiguousarray(wvec, np.float32),
            }
        )
    return in_maps


def gather_outputs(results):
    att = np.empty((B, CV, HW), np.float32)
    fqo = np.empty((B, CV, HW), np.float32)
    for k in range(NCORES):
        b, j = divmod(k, PSH)
        sl = slice(j * P, (j + 1) * P)
        att[b][:, sl] = results[k]["att_o"]
        fqo[b][:, sl] = results[k]["fq_o"]
    return (
        fqo.reshape(B, CV, H, W),
        att.reshape(B, CV, H, W),
    )


def kernel(fq_l3, fs_l3, fq_l4, fs_l4, f_q, f_s, w_red, trace=False):
    nc = build()
    in_maps = make_in_maps(fq_l3, fs_l3, fq_l4, fs_l4, f_q, f_s, w_red)
    res = run_bass_kernel_spmd(nc, in_maps, core_ids=list(range(NCORES)), trace=trace)
    out = gather_outputs(res.results)
    if trace:
        return out, res
    return out


# revision 24
# speedup vs baseline: 2.1644x; 1.0002x over previous
"""Trainium2 Bass kernel for nn_MMN_7361573945989 (MatchNet corr/attention).

Math (per batch b):
  qn_l = l2norm_c(fq_l); sn_l = l2norm_c(fs_l)           l in {4, 3}
  logits[p, q] = TEMP * (w0 * qn4.T@sn4 + w1 * qn3.T@sn3)[p, q]
  attn = softmax_q(logits)
  att_fq[c, p] = sum_q attn[p, q] * f_s[c, q]
  fq_out = l2norm_c(f_q) + l2norm_c(att_fq) * ATT_WT
  returns (fq_out, att_fq)

Sharding: 8 cores = 2 batches x 4 query-pixel shards of 900.

Per-core kernel (transposed orientation, logits live as [q, p] tiles):
  - all features arrive bf16 from the host; f_s pre-transposed to [hw, cv]
  - query side (P0): squares on DVE (bf16 4x), channel sums via ones-column
    matmuls into [1, PB] PSUM rows, broadcast via K=1 matmul, and the
    inverse norm (scaled by |TEMP*w_l|) comes from one activation:
    Exp(-0.5*Ln(ss) + ln|T*w_l|).  Sign of w_l folds into the query scaling.
  - support side (main loop, per 128-pixel chunk): squares on DVE, then 24
    single-column matmuls (lhsT = squared chunk, rhs = ones) contract the
    channel partition dim directly into per-layer [q, 1] PSUM columns; the
    inverse norms are per-partition [128,1] scalars -- no broadcasts.
  - logits accumulate per layer in separate PSUM groups (ps4 double-
    buffered across chunks, ps3 single), and the softmax exp is split as
    exp(l4 + l3) = Exp(ps4 * inv4[q]) * Exp(ps3 * inv3[q]) using the
    activation's per-partition scale operand; the product is one cheap
    bf16 DVE multiply into the persistent exp table.
  - only Exp/Ln/Square/Copy activations are used -> a single activation
    table ('natural_log_exp_and_others'), no table reloads.
  - phase B: two passes of 2 channel blocks; Y[c,p] accumulates exp @ f_s.T
    over chunks; softmax denominators via ones-matmuls in pass 0; the
    att_fq l2norm uses ||Y|| so the denominator cancels.
"""

import math
import sys
from contextlib import ExitStack

import numpy as np
import ml_dtypes

sys.path.insert(0, "/opt/trn_rl_repo")

import concourse.bass as bass  # noqa: E402
import concourse.tile as tile  # noqa: E402
from concourse import mybir  # noqa: E402
from concourse.bass_utils import run_bass_kernel_spmd  # noqa: E402

B, H, W = 2, 60, 60
HW = H * W  # 3600
C3, C4, CV = 1024, 2048, 512
TEMP = 20.0
ATT_WT = 0.3
NCORES = 8
PSH = 4  # query-pixel shards per batch
P = HW // PSH  # 900 query pixels per core
PB = P // 2  # 450, p-block (one PSUM bank of fp32)
NQC = (HW + 127) // 128  # 29 support-pixel chunks
QT = HW - (NQC - 1) * 128  # 16 rows in the tail chunk
NC4, NC3, NCV = C4 // 128, C3 // 128, CV // 128  # 16, 8, 4
NCI = NC4 + NC3  # 24 combined channel chunks

F32 = mybir.dt.float32
BF16 = mybir.dt.bfloat16
F8 = mybir.dt.float8e4
AF = mybir.ActivationFunctionType
MUL = mybir.AluOpType.mult
DR = mybir.MatmulPerfMode.DoubleRow

_MAX_WAITS_PER_INST = 1


def _patched_drain_and_barrier(self, tick_clock, wait_clock):
    """Tile's kernel-tail drain carries one sem wait per engine/queue; the
    walrus build used here accepts only one sync wait per CTRL instruction.
    Split the waits across extra sync-engine nops."""
    drain_inst = self.nc.sync.drain()
    wait_clock.add_sem_waits(
        drain_inst.ins, tile.ScopedClock({None: tick_clock.global_clock})
    )
    si = drain_inst.ins.sync_info
    if si is not None and len(si.on_wait) > _MAX_WAITS_PER_INST:
        waits = list(si.on_wait)
        drain_inst.ins.sync_info = mybir.SyncInfo(
            on_wait=waits[:_MAX_WAITS_PER_INST], on_update=list(si.on_update)
        )
        for i in range(_MAX_WAITS_PER_INST, len(waits), _MAX_WAITS_PER_INST):
            nop = self.nc.sync.nop()
            nop.ins.sync_info = mybir.SyncInfo(
                on_wait=waits[i : i + _MAX_WAITS_PER_INST], on_update=[]
            )
    self.nc.all_engine_barrier()
    assert self.sems is not None
    popped = self.nc._tile_sem_poison_stack.pop()
    assert popped is self._sem_poison
    self.nc.clear_and_free_semaphores(list(self.sems.allocated().values()))
    self.nc.all_engine_barrier()


tile.TileContext._drain_and_barrier = _patched_drain_and_barrier


def _split_sync_waits(nc, max_waits=_MAX_WAITS_PER_INST):
    """Walrus here accepts at most one sync wait per instruction; move excess
    waits onto same-engine nops inserted immediately before the instruction."""
    ctr = 0
    for f in nc.m.functions:
        for blk in f.blocks:
            insts = list(blk.instructions)
            out = []
            changed = False
            for inst in insts:
                si = inst.sync_info
                if si is not None and len(si.on_wait) > max_waits:
                    waits = list(si.on_wait)
                    for i0 in range(max_waits, len(waits), max_waits):
                        ctr += 1
                        nop = mybir.InstNoOp(
                            name=f"waitsplit-{ctr}",
                            engine=inst.engine,
                            bass_nofuse=True,
                            sync_info=mybir.SyncInfo(
                                on_wait=waits[i0 : i0 + max_waits], on_update=[]
                            ),
                        )
                        nc.register_instruction(nop, overwrite=True)
                        out.append(nop)
                    inst.sync_info = mybir.SyncInfo(
                        on_wait=waits[:max_waits], on_update=list(si.on_update)
                    )
                    changed = True
                out.append(inst)
            if changed:
                blk.instructions = out


def build():
    nc = bass.Bass()
    q4 = nc.dram_tensor("q4", [C4, P], BF16, kind="ExternalInput")
    q3 = nc.dram_tensor("q3", [C3, P], BF16, kind="ExternalInput")
    s4 = nc.dram_tensor("s4", [C4, HW], F8, kind="ExternalInput")
    s4b = nc.dram_tensor("s4b", [C4, HW], BF16, kind="ExternalInput")
    s3 = nc.dram_tensor("s3", [C3, HW], BF16, kind="ExternalInput")
    vt = nc.dram_tensor("vt", [HW, CV], BF16, kind="ExternalInput")  # f_s.T
    fq = nc.dram_tensor("fq", [CV, P], BF16, kind="ExternalInput")
    # wv = [[ln|T*w0|, ln|T*w1|]]
    wv = nc.dram_tensor("wv", [1, 2], F32, kind="ExternalInput")
    att_o = nc.dram_tensor("att_o", [CV, P], F32, kind="ExternalOutput")
    fq_o = nc.dram_tensor("fq_o", [CV, P], F32, kind="ExternalOutput")

    def load_blocks(dst, dst_cols, ci0, src, col0, ncols, n_ci, group=8, eng=None):
        """Load `n_ci` row-blocks of 128 from DRAM `src` (cols [col0,col0+ncols))
        into SBUF tile `dst` whose free layout is (ci, dst_cols)."""
        eng = eng or nc.sync
        srcr = src[:].rearrange("(ci c) x -> c ci x", c=128)
        dstr = dst[:].rearrange("c (ci x) -> c ci x", x=dst_cols)
        for g0 in range(0, n_ci, group):
            g = min(group, n_ci - g0)
            eng.dma_start(
                dstr[:, ci0 + g0 : ci0 + g0 + g, 0:ncols],
                srcr[:, g0 : g0 + g, col0 : col0 + ncols],
            )

    with tile.TileContext(nc) as tc:
        with ExitStack() as octx:
            cpool = octx.enter_context(tc.tile_pool(name="const", bufs=1))
            ones_col = cpool.tile([128, 1], BF16)
            nc.gpsimd.memset(ones_col[:], 1.0)
            ones8 = cpool.tile([128, 2], F8)  # k-tile pair of ones columns
            nc.gpsimd.memset(ones8[:], 1.0)
            ones_row = cpool.tile([1, 128], F32)
            nc.gpsimd.memset(ones_row[:], 1.0)
            w_sb = cpool.tile([1, 2], F32)
            nc.sync.dma_start(w_sb[:], wv[:])
            w_col = cpool.tile([128, 2], F32)
            lnw_att = cpool.tile([128, 1], F32)
            nc.gpsimd.memset(lnw_att[:], float(math.log(ATT_WT)))

            pers = octx.enter_context(tc.tile_pool(name="pers", bufs=1))
            qns4 = pers.tile([128, NC4 * P], F8)  # scaled query l4 (ci, p)
            qns3 = pers.tile([128, NC3 * P], BF16)  # scaled query l3 (ci, p)
            fqn = pers.tile([128, NCV * P], BF16)  # normalized f_q (ci, p)
            expT = pers.tile([128, NQC * P], BF16)  # exp(logits) (qc; q, p)
            vt_all = pers.tile([128, NQC * CV], BF16)  # f_s.T chunks (qc; q, c)
            # zero the tail-chunk regions so K=128 matmuls over the tail are
            # exact (rows [0:QT] get real data later)
            nc.gpsimd.memset(expT[:, (NQC - 1) * P : NQC * P], 0.0)
            nc.gpsimd.memset(vt_all[:, (NQC - 1) * CV : NQC * CV], 0.0)

            # broadcast ln|T*w| across partitions once: [1,2] -> [128,2]
            with tc.tile_pool(name="wps", bufs=1, space="PSUM") as wps:
                w_ps = wps.tile([128, 2], F32)
                nc.tensor.matmul(w_ps[:], ones_row[:], w_sb[:])
                nc.scalar.copy(w_col[:], w_ps[:])

            # ---------------- P0: query-side normalization ----------------
            # src tile (raw bf16), dst tile, n_ci, dram src, w-col index
            # (None => no |T*w|); sign(w_l) is folded on the host
            with ExitStack() as pctx:
                sqp = pctx.enter_context(tc.tile_pool(name="p0sq", bufs=2))
                mini = pctx.enter_context(tc.tile_pool(name="p0mini", bufs=2))
                stg = pctx.enter_context(tc.tile_pool(name="p0stg", bufs=1))
                pps = pctx.enter_context(
                    tc.tile_pool(name="p0ps", bufs=1, space="PSUM")
                )
                q4s = stg.tile([128, NC4 * P], BF16)  # raw q4 staging
                # layer 3 first: its chain is short, so the main loop's
                # layer-3 matmuls (emitted first per chunk) start earliest
                layers = [
                    (qns3, qns3, NC3, q3, 1),
                    (q4s, qns4, NC4, q4, 0),
                    (fqn, fqn, NCV, fq, None),
                ]
                for li, (raw, dst, n_ci, src, wl) in enumerate(layers):
                    load_blocks(raw, P, 0, src, 0, P, n_ci, group=4)
                    ss = [
                        pps.tile([1, PB], F32, tag=f"ss{li}_{pb}", name=f"ss{li}_{pb}")
                        for pb in range(2)
                    ]
                    for k in range(n_ci):
                        xg = raw[:, k * P : (k + 1) * P]
                        sq = sqp.tile([128, P], BF16, tag="sq", name=f"sqq{li}_{k}")
                        nc.vector.tensor_mul(sq[:], xg, xg)
                        for pb in range(2):
                            nc.tensor.matmul(
                                ss[pb][:],
                                ones_col[:],
                                sq[:, pb * PB : (pb + 1) * PB],
                                start=(k == 0),
                                stop=(k == n_ci - 1),
                            )
                    invqs = []
                    for pb in range(2):
                        u = mini.tile([1, PB], F32, tag="u", name=f"u{li}_{pb}")
                        nc.scalar.copy(u[:], ss[pb][:])
                        bc = pps.tile(
                            [128, PB], F32, tag="bc", name=f"bc{li}_{pb}", bufs=2
                        )
                        nc.tensor.matmul(bc[:], ones_row[:], u[:])
                        lnb = mini.tile(
                            [128, PB], F32, tag="lnb", name=f"lnb{li}_{pb}"
                        )
                        nc.scalar.activation(lnb[:], bc[:], AF.Ln)
                        invq = mini.tile(
                            [128, PB], BF16, tag="invq", name=f"invq{li}_{pb}"
                        )
                        # Exp(-0.5*ln(ss) + ln|T*w|) = |T*w| / sqrt(ss)
                        if wl is None:
                            nc.scalar.activation(
                                invq[:], lnb[:], AF.Exp, scale=-0.5
                            )
                        else:
                            nc.scalar.activation(
                                invq[:],
                                lnb[:],
                                AF.Exp,
                                scale=-0.5,
                                bias=w_col[:, wl : wl + 1],
                            )
                        invqs.append(invq)
                    # k-outer so the first channel chunks are ready for the
                    # main loop's first matmuls as early as possible
                    for k in range(n_ci):
                        for pb in range(2):
                            sl = slice(k * P + pb * PB, k * P + pb * PB + PB)
                            nc.vector.tensor_mul(
                                dst[:, sl], raw[:, sl], invqs[pb][:]
                            )

            # ------------- main: support stream, logits, exp -------------
            with ExitStack() as mctx:
                snpool = mctx.enter_context(tc.tile_pool(name="sn", bufs=3))
                sqpool = mctx.enter_context(tc.tile_pool(name="msq", bufs=2))
                mmini = mctx.enter_context(tc.tile_pool(name="mmini", bufs=2))
                epool = mctx.enter_context(tc.tile_pool(name="etmp", bufs=2))
                mps = mctx.enter_context(
                    tc.tile_pool(name="mps", bufs=1, space="PSUM")
                )

                qns4r = qns4[:].rearrange("c (ci p) -> c ci p", p=P)
                ones8r = ones8[:].rearrange("c (k f) -> c k f", f=1)
                for qc in range(NQC):
                    qn = 128 if qc < NQC - 1 else QT
                    sn3 = snpool.tile(
                        [128, NC3 * 128], BF16, tag="sn3", name=f"sn3_{qc}"
                    )
                    sn4 = snpool.tile(
                        [128, NC4 * 128], F8, tag="sn4", name=f"sn4_{qc}"
                    )
                    load_blocks(sn3, 128, 0, s3, qc * 128, qn, NC3, eng=nc.gpsimd)
                    load_blocks(sn4, 128, 0, s4, qc * 128, qn, NC4, eng=nc.gpsimd)
                    nc.gpsimd.dma_start(
                        vt_all[0:qn, qc * CV : (qc + 1) * CV],
                        vt[qc * 128 : qc * 128 + qn, :],
                    )

                    # squares (for support norms) as fp8: feeds DoubleRow
                    # ones-matmuls, halving the LDWEIGHTS byte traffic
                    sq4 = sqpool.tile([128, NC4 * 128], F8, tag="sq4", name=f"sq4_{qc}")
                    sq3 = sqpool.tile([128, NC3 * 128], F8, tag="sq3", name=f"sq3_{qc}")
                    nc.vector.tensor_mul(sq4[:], sn4[:], sn4[:])
                    nc.vector.tensor_mul(sq3[:], sn3[:], sn3[:])

                    # logits: per-layer PSUM groups; layer 3 (bf16) first,
                    # layer 4 fp8 DoubleRow (K=256 per instruction at 2x rate)
                    ps3 = [
                        mps.tile(
                            [128, PB], F32, tag=f"ps3_{pb}", name=f"ps3_{pb}_{qc}",
                            bufs=2,
                        )
                        for pb in range(2)
                    ]
                    ps4 = [
                        mps.tile(
                            [128, PB], F32, tag=f"ps4_{pb}", name=f"ps4_{pb}_{qc}"
                        )
                        for pb in range(2)
                    ]
                    for k in range(NC3):
                        lhsT = sn3[:, k * 128 : k * 128 + qn]
                        for pb in range(2):
                            nc.tensor.matmul(
                                ps3[pb][0:qn, :],
                                lhsT,
                                qns3[:, k * P + pb * PB : k * P + pb * PB + PB],
                                start=(k == 0),
                                stop=(k == NC3 - 1),
                            )
                    sn4r = sn4[:].rearrange("c (ci q) -> c ci q", q=128)
                    for i in range(NC4 // 2):
                        lhsT = sn4r[:, 2 * i : 2 * i + 2, 0:qn]
                        for pb in range(2):
                            nc.tensor.matmul(
                                ps4[pb][0:qn, :],
                                lhsT,
                                qns4r[
                                    :, 2 * i : 2 * i + 2,
                                    pb * PB : pb * PB + PB,
                                ],
                                start=(i == 0),
                                stop=(i == NC4 // 2 - 1),
                                perf_mode=DR,
                            )
                    # support norms (after logits so the PE never waits on the
                    # fresh chunk's squares): contract the channel partition
                    # dim into per-layer [q, 1] PSUM columns via fp8 DoubleRow
                    # ones-matmuls; layer 4 first (its inverse gates exp4,
                    # which reads the single-buffered ps4)
                    sscols = [
                        mps.tile([128, 1], F32, tag="ss4", name=f"ss4_{qc}"),
                        mps.tile([128, 1], F32, tag="ss3", name=f"ss3_{qc}"),
                    ]
                    sq4r = sq4[:].rearrange("c (ci q) -> c ci q", q=128)
                    sq3r = sq3[:].rearrange("c (ci q) -> c ci q", q=128)
                    for l, sqr, npair in ((0, sq4r, NC4 // 2), (1, sq3r, NC3 // 2)):
                        for i in range(npair):
                            nc.tensor.matmul(
                                sscols[l][0:qn, :],
                                sqr[:, 2 * i : 2 * i + 2, 0:qn],
                                ones8r[:],
                                start=(i == 0),
                                stop=(i == npair - 1),
                                perf_mode=DR,
                            )
                    invc = []
                    for l in range(2):
                        lns = mmini.tile(
                            [128, 1], F32, tag=f"lns{l}", name=f"lns{l}_{qc}"
                        )
                        nc.scalar.activation(
                            lns[0:qn, :], sscols[l][0:qn, :], AF.Ln
                        )
                        inv = mmini.tile(
                            [128, 1], F32, tag=f"inv{l}", name=f"inv{l}_{qc}"
                        )
                        nc.scalar.activation(
                            inv[0:qn, :], lns[0:qn, :], AF.Exp, scale=-0.5
                        )
                        invc.append(inv)
                    # exp4 first: it reads the single-buffered ps4 banks
                    e4s = []
                    for pb in range(2):
                        e4 = epool.tile(
                            [128, PB], BF16, tag=f"e4_{pb}", name=f"e4_{pb}_{qc}"
                        )
                        nc.scalar.activation(
                            e4[0:qn, :],
                            ps4[pb][0:qn, :],
                            AF.Exp,
                            scale=invc[0][0:qn, :],
                        )
                        e4s.append(e4)
                    for pb in range(2):
                        e3 = epool.tile(
                            [128, PB], BF16, tag=f"e3_{pb}", name=f"e3_{pb}_{qc}"
                        )
                        nc.scalar.activation(
                            e3[0:qn, :],
                            ps3[pb][0:qn, :],
                            AF.Exp,
                            scale=invc[1][0:qn, :],
                        )
                        nc.vector.tensor_mul(
                            expT[0:qn, qc * P + pb * PB : qc * P + pb * PB + PB],
                            e4s[pb][0:qn, :],
                            e3[0:qn, :],
                        )

            # ---------------- phase B: attention-weighted values ----------------
            with ExitStack() as bctx:
                bps = bctx.enter_context(
                    tc.tile_pool(name="bps", bufs=1, space="PSUM")
                )
                bmini = bctx.enter_context(tc.tile_pool(name="bmini", bufs=1))
                bsq = bctx.enter_context(tc.tile_pool(name="bsq", bufs=2))
                batt = bctx.enter_context(tc.tile_pool(name="batt", bufs=1))
                bout = bctx.enter_context(tc.tile_pool(name="bout", bufs=2))

                ssy = [
                    bps.tile([1, PB], F32, tag=f"ssy{pb}", name=f"ssy{pb}")
                    for pb in range(2)
                ]
                dns = [
                    bps.tile([1, PB], F32, tag=f"dn{pb}", name=f"dn{pb}")
                    for pb in range(2)
                ]
                att_sb = {}
                inv_dn, raw_dn = [], []
                # denominators first: their matmuls are cheap and stopping the
                # accumulation early lets the 1/dn chain overlap the Y matmuls
                for qc in range(NQC):
                    for pb in range(2):
                        nc.tensor.matmul(
                            dns[pb][:],
                            ones_col[:],
                            expT[:, qc * P + pb * PB : qc * P + pb * PB + PB],
                            start=(qc == 0),
                            stop=(qc == NQC - 1),
                        )
                for pb in range(2):
                    u = bmini.tile([1, PB], F32, tag=f"ud{pb}", name=f"ud{pb}")
                    nc.scalar.copy(u[:], dns[pb][:])
                    bcp = bps.tile([128, PB], F32, tag=f"dn{pb}", name=f"bd{pb}")
                    nc.tensor.matmul(bcp[:], ones_row[:], u[:])
                    raw = bmini.tile(
                        [128, PB], F32, tag=f"dnraw{pb}", name=f"dnraw{pb}"
                    )
                    nc.scalar.copy(raw[:], bcp[:])
                    inv = bmini.tile(
                        [128, PB], F32, tag=f"dninv{pb}", name=f"dninv{pb}"
                    )
                    nc.vector.reciprocal(inv[:], bcp[:])
                    inv_dn.append(inv)
                    raw_dn.append(raw)
                for pss in range(2):
                    ys = {}
                    for cbk in range(2):
                        cb = 2 * pss + cbk
                        for pb in range(2):
                            ys[(cb, pb)] = bps.tile(
                                [128, PB], F32, tag=f"y{cbk}_{pb}",
                                name=f"y{cb}_{pb}",
                            )
                    for qc in range(NQC):
                        for cbk in range(2):
                            cb = 2 * pss + cbk
                            lhsT = vt_all[
                                :, qc * CV + cb * 128 : qc * CV + (cb + 1) * 128
                            ]
                            for pb in range(2):
                                nc.tensor.matmul(
                                    ys[(cb, pb)][:],
                                    lhsT,
                                    expT[:, qc * P + pb * PB : qc * P + pb * PB + PB],
                                    start=(qc == 0),
                                    stop=(qc == NQC - 1),
                                )
                    for cbk in range(2):
                        cb = 2 * pss + cbk
                        for pb in range(2):
                            att = batt.tile(
                                [128, PB], F32, tag=f"att{cb}_{pb}",
                                name=f"att{cb}_{pb}",
                            )
                            nc.vector.tensor_mul(
                                att[:], ys[(cb, pb)][:], inv_dn[pb][:]
                            )
                            att_sb[(cb, pb)] = att
                            nc.sync.dma_start(
                                att_o[
                                    cb * 128 : (cb + 1) * 128,
                                    pb * PB : (pb + 1) * PB,
                                ],
                                att[:],
                            )
                            sqy = bsq.tile([128, PB], BF16, tag="sqy")
                            nc.scalar.square(sqy[:], ys[(cb, pb)][:])
                            nc.tensor.matmul(
                                ssy[pb][:],
                                ones_col[:],
                                sqy[:],
                                start=(cb == 0),
                                stop=(cb == NCV - 1),
                            )

                for pb in range(2):
                    u = bmini.tile([1, PB], F32, tag=f"us{pb}", name=f"us{pb}")
                    nc.scalar.copy(u[:], ssy[pb][:])
                    bcp = bps.tile([128, PB], F32, tag=f"ssy{pb}", name=f"bs{pb}")
                    nc.tensor.matmul(bcp[:], ones_row[:], u[:])
                    lnb = bmini.tile([128, PB], F32, tag=f"lnbs{pb}", name=f"lnbs{pb}")
                    nc.scalar.activation(lnb[:], bcp[:], AF.Ln)
                    # Exp(-0.5*ln(ssy) + ln(0.3)) = 0.3/||Y||
                    sinv = bmini.tile([128, PB], F32, tag=f"sinv{pb}", name=f"sinv{pb}")
                    nc.scalar.activation(
                        sinv[:], lnb[:], AF.Exp, scale=-0.5, bias=lnw_att[:]
                    )
                    # fq = fqn + att * (denom * 0.3/||Y||)
                    s2 = bmini.tile([128, PB], F32, tag=f"s2{pb}", name=f"s2{pb}")
                    nc.vector.tensor_mul(s2[:], raw_dn[pb][:], sinv[:])
                    for cb in range(NCV):
                        t = bout.tile([128, PB], F32, tag="t")
                        nc.vector.tensor_mul(t[:], att_sb[(cb, pb)][:], s2[:])
                        f_sb = bout.tile([128, PB], F32, tag="f")
                        nc.vector.tensor_add(
                            f_sb[:],
                            t[:],
                            fqn[:, cb * P + pb * PB : cb * P + pb * PB + PB],
                        )
                        nc.sync.dma_start(
                            fq_o[cb * 128 : (cb + 1) * 128, pb * PB : (pb + 1) * PB],
                            f_sb[:],
                        )
    _split_sync_waits(nc)
    return nc


def make_in_maps(fq_l3, fs_l3, fq_l4, fs_l4, f_q, f_s, w_red):
    bf = ml_dtypes.bfloat16
    wr = np.asarray(w_red, np.float32)
    wvec = np.log(np.abs(TEMP * wr)).reshape(1, 2)
    # fold sign(w_l) into the query features; |T*w_l| rides in wvec
    q4f = float(np.sign(wr[0])) * np.asarray(fq_l4, np.float32).reshape(B, C4, HW)
    q3f = float(np.sign(wr[1])) * np.asarray(fq_l3, np.float32).reshape(B, C3, HW)
    s4f = np.asarray(fs_l4, np.float32).reshape(B, C4, HW)
    s3f = np.asarray(fs_l3, np.float32).reshape(B, C3, HW)
    vf = np.asarray(f_s, np.float32).reshape(B, CV, HW)
    fqf = np.asarray(f_q, np.float32).reshape(B, CV, HW)
    f8 = ml_dtypes.float8_e4m3
    s4b = [np.ascontiguousarray(s4f[b]).astype(bf) for b in range(B)]
    s48 = [x.astype(f8) for x in s4b]
    s3b = [np.ascontiguousarray(s3f[b]).astype(bf) for b in range(B)]
    vtb = [np.ascontiguousarray(vf[b].T).astype(bf) for b in range(B)]
    in_maps = []
    for k in range(NCORES):
        b, j = divmod(k, PSH)
        sl = slice(j * P, (j + 1) * P)
        in_maps.append(
            {
                "q4": np.ascontiguousarray(q4f[b][:, sl]).astype(bf),
                "q3": np.ascontiguousarray(q3f[b][:, sl]).astype(bf),
                "s4": s48[b],
                "s4b": s4b[b],
                "s3": s3b[b],
                "vt": vtb[b],
                "fq": np.ascontiguousarray(fqf[b][:, sl]).astype(bf),
                "wv": np.ascontiguousarray(wvec, np.float32),
            }
        )
    return in_maps


def gather_outputs(results):
    att = np.empty((B, CV, HW), np.float32)
    fqo = np.empty((B, CV, HW), np.float32)
    for k in range(NCORES):
        b, j = divmod(k, PSH)
        sl = slice(j * P, (j + 1) * P)
        att[b][:, sl] = results[k]["att_o"]
        fqo[b][:, sl] = results[k]["fq_o"]
    return (
        fqo.reshape(B, CV, H, W),
        att.reshape(B, CV, H, W),
    )


def kernel(fq_l3, fs_l3, fq_l4, fs_l4, f_q, f_s, w_red, trace=False):
    nc = build()
    in_maps = make_in_maps(fq_l3, fs_l3, fq_l4, fs_l4, f_q, f_s, w_red)
    res = run_bass_kernel_spmd(nc, in_maps, core_ids=list(range(NCORES)), trace=trace)
    out = gather_outputs(res.results)
    if trace:
        return out, res
    return out
